# revision 5
# baseline (speedup 1.0000x reference)
"""GCMC graph-conv kernel for Trainium2, 8-core SPMD.

out = ci * segment_sum((weight[node_ids] * cj)[src_idx], dst_idx)

Strategy (edge sharding by dst range, fp16 message path):
  - host prescales W' = weight[node_ids] * cj, stores it as an fp16 table with
    256B-strided rows ([100000, 128] fp16, data in cols 0:64) so each gather
    descriptor moves only 128B (half the DMA time of a 256B fp32 row)
  - core k owns dst rows [k*12500, (k+1)*12500); its edges are partitioned by
    (supertile of G=7 dst tiles, src chunk of 25000, dst tile) with each
    (supertile, chunk, tile) sub-segment padded to 128 slots using a shared
    static envelope (max over cores) so the program is SPMD-identical
  - one SWDGE dma_gather per (supertile, chunk) — 56 gathers instead of 392 —
    emitted raw (the bass wrapper's elem%256 assert is a transpose-path
    restriction; elem_step=128/elem_size=64 fp16 is valid and verified on hw)
  - segment-sum via one-hot matmul: oh[slot, d] = (iota[d] == dv[slot]) built
    on DVE (tensor_scalar is_equal) for ~3/4 of blocks and on ACT
    (t=Abs(io-dv); oh=Relu(1-t)) for the rest to split the one-hot cost
    across both engines; PE accumulates psum[dst,64] += oh.T @ msg in fp16
  - flush: ACT copies psum*ci into a per-supertile staging tile, one HWDGE
    DMA per supertile writes [128, G*64] fp32 to a partition-major output
    buffer that the host untransposes
"""
import sys, os
sys.path.insert(0, '/opt/trn_rl_repo')

import numpy as np

N_NODES = 100000
OUT_DIM = 64
N_CORES = 8
DST_PER_CORE = N_NODES // N_CORES          # 12500
N_TILES = (DST_PER_CORE + 127) // 128      # 98
G = 7                                      # dst tiles per supertile
N_SUPER = N_TILES // G                     # 14
N_CHUNKS = 4                               # int16 idx -> <=25000 rows per chunk
CHUNK = N_NODES // N_CHUNKS                # 25000
PAD_SENTINEL = 999.0
POOL_FRACTION = 8                          # every 8th block's one-hot on GPSIMD


def _round_up(x, m):
    return (x + m - 1) // m * m


def _host_prep(src, dst):
    """Partition edges by dst core range; compute the shared static envelope
    env[s, c, t] (max per-core (supertile, chunk, tile) count, rounded to 128)
    and per-core slot-packed idx / dv arrays laid out in envelope slots."""
    per_core = []
    counts = np.zeros((N_CORES, N_SUPER, N_CHUNKS, G), np.int64)
    for k in range(N_CORES):
        m = (dst // DST_PER_CORE) == k
        s_e = src[m]
        dl = dst[m] - k * DST_PER_CORE
        t = dl >> 7                         # dst tile 0..97
        sg = t // G                         # supertile 0..13
        tl = t - sg * G                     # tile within supertile 0..6
        c = s_e // CHUNK                    # src chunk 0..3
        order = np.lexsort((tl, c, sg))
        s_e, dl, tl, c, sg = s_e[order], dl[order], tl[order], c[order], sg[order]
        grp = (sg * N_CHUNKS + c) * G + tl
        counts[k] = np.bincount(grp, minlength=N_SUPER * N_CHUNKS * G).reshape(
            N_SUPER, N_CHUNKS, G)
        per_core.append((s_e, dl, grp))

    env = _round_up(counts.max(axis=0), 128)       # [S, C, G]
    env_flat = env.reshape(-1)
    slot_off = np.concatenate([[0], np.cumsum(env_flat)])[:-1]
    total = int(env_flat.sum())                    # total slots (mult of 128)

    idx_all, dv_all = [], []
    for k in range(N_CORES):
        s_e, dl, grp = per_core[k]
        grp_counts = np.bincount(grp, minlength=N_SUPER * N_CHUNKS * G)
        within = np.arange(len(s_e)) - np.repeat(
            np.concatenate([[0], np.cumsum(grp_counts)])[:-1], grp_counts)
        slot = slot_off[grp] + within
        c_of = grp // G % N_CHUNKS
        idx_flat = np.zeros(total, np.int16)       # pad slots gather row 0
        idx_flat[slot] = (s_e - c_of * CHUNK).astype(np.int16)
        dv_flat = np.full(total, PAD_SENTINEL, np.float32)
        dv_flat[slot] = (dl & 127).astype(np.float32)
        # idx wrapped into 16 partitions, replicated x8 (one copy per Q7 core)
        idx_all.append(np.tile(idx_flat.reshape(total // 16, 16).T, (8, 1)).copy())
        dv_all.append(dv_flat.reshape(total // 128, 128).T.copy())
    return env, slot_off, total, idx_all, dv_all


def _raw_dma_gather(gp, out_ap, in_ap, idxs_ap, num_idxs, elem_size, elem_step):
    """dma_gather without the elem_size_bytes%256 assert (transpose-path-only
    restriction). Rows are elem_step-strided; each descriptor moves elem_size
    elements. Verified bit-exact on hardware for fp16 elem 64 / step 128."""
    import concourse.mybir as mybir
    import concourse.ap_utils as ap_utils
    assert in_ap.dtype == out_ap.dtype
    assert idxs_ap.dtype == mybir.dt.int16
    assert ap_utils.ap_is_contiguous(in_ap.ap[1:])
    assert ap_utils.ap_is_contiguous(out_ap.ap[1:])
    assert ap_utils.ap_is_contiguous(idxs_ap.ap[1:])
    assert in_ap.ap[-1][1] == out_ap.ap[-1][1] == elem_size
    assert in_ap.ap[0][0] == elem_step
    assert out_ap.ap[0][1] * out_ap.ap[1][1] == _round_up(num_idxs, 128)
    stride_bytes = elem_step * mybir.dt.size(in_ap.dtype)
    assert stride_bytes % 256 == 0 and stride_bytes // 256 < 256
    _in_ap = gp.lower_ap_dma(in_ap, for_custom_bir_dma=True)
    return gp.add_instruction(
        mybir.InstDMAGatherAnt(
            name=gp.bass.get_next_instruction_name(),
            ins=[*_in_ap, gp.lower_ap(idxs_ap),
                 gp.lower_val_access(gp.to_reg(num_idxs))],
            outs=[gp.lower_ap(out_ap)],
            transpose=False,
            num_idxs=num_idxs,
            elem_size=elem_size,
            stride_bytes_256=stride_bytes // 256,
            gen_mode=0,
            single_packet=False,
            queue_num=0,
        )
    )


def _build_program(env, slot_off, total):
    import concourse.bass as bass
    import concourse.bacc as bacc
    import concourse.mybir as mybir
    import concourse.tile as tile

    f32 = mybir.dt.float32
    fp16 = mybir.dt.float16
    n_blocks = total // 128
    seg_slots = env.sum(axis=2)                    # [S, C] slots per gather
    max_seg_blocks = int(seg_slots.max()) // 128

    nc = bacc.Bacc("TRN2", target_bir_lowering=False, debug=False,
                   num_devices=N_CORES)
    w_d = nc.dram_tensor("w", [N_NODES, 128], fp16, kind="ExternalInput").ap()
    ci_d = nc.dram_tensor("ci", [128, N_TILES], f32, kind="ExternalInput").ap()
    io_d = nc.dram_tensor("io", [128, 128], fp16, kind="ExternalInput").ap()
    idx_d = nc.dram_tensor("idx", [128, total // 16], mybir.dt.int16,
                           kind="ExternalInput").ap()
    dv_d = nc.dram_tensor("dv", [128, n_blocks], f32, kind="ExternalInput").ap()
    out_d = nc.dram_tensor("out", [128, N_TILES * OUT_DIM], f32,
                           kind="ExternalOutput").ap()

    with tile.TileContext(nc) as tc:
        with (
            tc.tile_pool(name="const", bufs=1) as constp,
            tc.tile_pool(name="msg", bufs=6) as msgp,
            tc.tile_pool(name="oh", bufs=8) as ohp,
            tc.tile_pool(name="ps", bufs=8, space="PSUM") as psp,
            tc.tile_pool(name="ot", bufs=2) as otp,
        ):
            ci_t = constp.tile([128, N_TILES], f32)
            io_t = constp.tile([128, 128], fp16)
            idx_t = constp.tile([128, total // 16], mybir.dt.int16)
            dv_t = constp.tile([128, n_blocks], f32)
            nc.sync.dma_start(ci_t[:], ci_d[:])
            nc.sync.dma_start(io_t[:], io_d[:])
            # chunk metadata uploads per supertile so early gathers/one-hots
            # only wait on their own slice
            for s in range(N_SUPER):
                a = int(slot_off[s * N_CHUNKS * G])
                b = int(slot_off[(s + 1) * N_CHUNKS * G]) if s + 1 < N_SUPER \
                    else total
                nc.sync.dma_start(idx_t[:, a // 16:b // 16],
                                  idx_d[:, a // 16:b // 16])
                nc.sync.dma_start(dv_t[:, a // 128:b // 128],
                                  dv_d[:, a // 128:b // 128])

            for s in range(N_SUPER):
                msgs = []
                for c in range(N_CHUNKS):
                    n_sc = int(seg_slots[s, c])
                    off = int(slot_off[(s * N_CHUNKS + c) * G])
                    msg = msgp.tile([128, max_seg_blocks, OUT_DIM], fp16,
                                    tag="msg")
                    _raw_dma_gather(
                        nc.gpsimd, msg[:, :n_sc // 128, :],
                        w_d[c * CHUNK:(c + 1) * CHUNK, 0:OUT_DIM],
                        idx_t[:, off // 16:(off + n_sc) // 16],
                        n_sc, OUT_DIM, 128)
                    msgs.append(msg)

                pss = [psp.tile([128, OUT_DIM], f32, tag="ps",
                                name=f"ps_{s}_{tl}")
                       for tl in range(G)]
                # per tile: count of blocks remaining (for start/stop flags)
                blk_total = [int(env[s, :, tl].sum()) // 128 for tl in range(G)]
                blk_seen = [0] * G
                for c in range(N_CHUNKS):
                    seg_base = int(slot_off[(s * N_CHUNKS + c) * G])
                    col = 0
                    for tl in range(G):
                        n_blk_t = int(env[s, c, tl]) // 128
                        for b in range(n_blk_t):
                            gcol = seg_base // 128 + col + b
                            oh = ohp.tile([128, 128], fp16, tag="oh")
                            eng = nc.gpsimd if gcol % POOL_FRACTION == 0 \
                                else nc.vector
                            eng.tensor_scalar(
                                oh[:], io_t[:], dv_t[:, gcol:gcol + 1],
                                1.0, mybir.AluOpType.is_equal,
                                mybir.AluOpType.mult)
                            nc.tensor.matmul(
                                pss[tl][:], oh[:], msgs[c][:, col + b, :],
                                start=(blk_seen[tl] == 0),
                                stop=(blk_seen[tl] == blk_total[tl] - 1))
                            blk_seen[tl] += 1
                        col += n_blk_t

                ot = otp.tile([128, G * OUT_DIM], f32, tag="ot")
                for tl in range(G):
                    t = s * G + tl
                    nc.scalar.activation(
                        ot[:, tl * OUT_DIM:(tl + 1) * OUT_DIM], pss[tl][:],
                        mybir.ActivationFunctionType.Copy,
                        scale=ci_t[:, t:t + 1])
                nc.sync.dma_start(
                    out_d[:, s * G * OUT_DIM:(s + 1) * G * OUT_DIM], ot[:])

    nc.compile()
    return nc


def prepare(node_ids, src_idx, dst_idx, cj, ci, weight):
    """Host prep + program build. Returns (nc, in_maps, postprocess)."""
    import time
    _t0 = time.time()

    node_ids = np.asarray(node_ids)
    src = np.asarray(src_idx).astype(np.int64)
    dst = np.asarray(dst_idx).astype(np.int64)
    cj = np.asarray(cj, dtype=np.float32).reshape(-1)
    ci = np.asarray(ci, dtype=np.float32).reshape(-1)
    weight = np.asarray(weight, dtype=np.float32)

    # feat rows are weight[node_ids]; with the arange fill this is identity
    if not np.array_equal(node_ids, np.arange(N_NODES, dtype=node_ids.dtype)):
        weight = weight[node_ids]

    # prescale by cj and lay out as an fp16 table with 256B-strided rows
    w_tab = np.zeros((N_NODES, 128), np.float16)
    w_tab[:, :OUT_DIM] = (weight * cj[:, None]).astype(np.float16)

    iota = np.tile(np.arange(128, dtype=np.float16), (128, 1))

    env, slot_off, total, idx_all, dv_all = _host_prep(src, dst)
    print(f"[kernel] host prep: {time.time()-_t0:.1f}s (total slots {total})",
          flush=True)
    _t1 = time.time()
    nc = _build_program(env, slot_off, total)
    print(f"[kernel] build+schedule+compile-to-bir: {time.time()-_t1:.1f}s",
          flush=True)

    in_maps = []
    for k in range(N_CORES):
        ci_k = np.zeros(N_TILES * 128, np.float32)
        ci_k[:DST_PER_CORE] = ci[k * DST_PER_CORE:(k + 1) * DST_PER_CORE]
        ci_w = ci_k.reshape(N_TILES, 128).T.copy()
        in_maps.append({
            "w": w_tab, "ci": ci_w, "io": iota,
            "idx": idx_all[k], "dv": dv_all[k],
        })

    def post(results):
        outs = []
        for k in range(N_CORES):
            o = np.asarray(results[k]["out"])        # [128, 98*64]
            o = o.reshape(128, N_TILES, OUT_DIM).transpose(1, 0, 2)
            outs.append(o.reshape(-1, OUT_DIM)[:DST_PER_CORE])
        return np.concatenate(outs, axis=0)

    return nc, in_maps, post


def kernel(node_ids, src_idx, dst_idx, cj, ci, weight):
    import time
    from concourse.bass_utils import run_bass_kernel_spmd
    nc, in_maps, post = prepare(node_ids, src_idx, dst_idx, cj, ci, weight)
    _t2 = time.time()
    res = run_bass_kernel_spmd(nc, in_maps, core_ids=list(range(N_CORES)))
    print(f"[kernel] neff compile+exec: {time.time()-_t2:.1f}s", flush=True)
    return post(res.results)


# revision 6
# speedup vs baseline: 1.0927x; 1.0927x over previous
"""GCMC graph-conv kernel for Trainium2, 8-core SPMD.

out = ci * segment_sum((weight[node_ids] * cj)[src_idx], dst_idx)

Strategy (edge sharding by dst range, fp16 message path):
  - host prescales W' = weight[node_ids] * cj, stores it as an fp16 table with
    256B-strided rows ([100000, 128] fp16, data in cols 0:64) so each gather
    descriptor moves only 128B (half the DMA time of a 256B fp32 row)
  - core k owns dst rows [k*12500, (k+1)*12500); its edges are partitioned by
    (supertile of G=7 dst tiles, src chunk of 25000, dst tile) with each
    (supertile, chunk, tile) sub-segment padded to 128 slots using a shared
    static envelope (max over cores) so the program is SPMD-identical
  - one SWDGE dma_gather per (supertile, chunk) — 56 gathers instead of 392 —
    emitted raw (the bass wrapper's elem%256 assert is a transpose-path
    restriction; elem_step=128/elem_size=64 fp16 is valid and verified on hw)
  - segment-sum via one-hot matmul: oh[slot, d] = (iota[d] == dv[slot]) built
    on DVE (tensor_scalar is_equal) for ~3/4 of blocks and on ACT
    (t=Abs(io-dv); oh=Relu(1-t)) for the rest to split the one-hot cost
    across both engines; PE accumulates psum[dst,64] += oh.T @ msg in fp16
  - flush: ACT copies psum*ci into a per-supertile staging tile, one HWDGE
    DMA per supertile writes [128, G*64] fp32 to a partition-major output
    buffer that the host untransposes
"""
import sys, os
sys.path.insert(0, '/opt/trn_rl_repo')

import numpy as np

N_NODES = 100000
OUT_DIM = 64
N_CORES = 8
DST_PER_CORE = N_NODES // N_CORES          # 12500
N_TILES = (DST_PER_CORE + 127) // 128      # 98
G = 7                                      # dst tiles per supertile
N_SUPER = N_TILES // G                     # 14
N_CHUNKS = 4                               # int16 idx -> <=25000 rows per chunk
CHUNK = N_NODES // N_CHUNKS                # 25000
PAD_SENTINEL = 999.0
POOL_FRACTION = 8                          # every 8th block's one-hot on GPSIMD


def _round_up(x, m):
    return (x + m - 1) // m * m


def _host_prep(src, dst):
    """Partition edges by dst core range; compute the shared static envelope
    env[s, c, t] (max per-core (supertile, chunk, tile) count, rounded to 128)
    and per-core slot-packed idx / dv arrays laid out in envelope slots."""
    per_core = []
    counts = np.zeros((N_CORES, N_SUPER, N_CHUNKS, G), np.int64)
    for k in range(N_CORES):
        m = (dst // DST_PER_CORE) == k
        s_e = src[m]
        dl = dst[m] - k * DST_PER_CORE
        t = dl >> 7                         # dst tile 0..97
        sg = t // G                         # supertile 0..13
        tl = t - sg * G                     # tile within supertile 0..6
        c = s_e // CHUNK                    # src chunk 0..3
        order = np.lexsort((tl, c, sg))
        s_e, dl, tl, c, sg = s_e[order], dl[order], tl[order], c[order], sg[order]
        grp = (sg * N_CHUNKS + c) * G + tl
        counts[k] = np.bincount(grp, minlength=N_SUPER * N_CHUNKS * G).reshape(
            N_SUPER, N_CHUNKS, G)
        per_core.append((s_e, dl, grp))

    env = _round_up(counts.max(axis=0), 128)       # [S, C, G]
    env_flat = env.reshape(-1)
    slot_off = np.concatenate([[0], np.cumsum(env_flat)])[:-1]
    total = int(env_flat.sum())                    # total slots (mult of 128)

    idx_all, dv_all = [], []
    for k in range(N_CORES):
        s_e, dl, grp = per_core[k]
        grp_counts = np.bincount(grp, minlength=N_SUPER * N_CHUNKS * G)
        within = np.arange(len(s_e)) - np.repeat(
            np.concatenate([[0], np.cumsum(grp_counts)])[:-1], grp_counts)
        slot = slot_off[grp] + within
        c_of = grp // G % N_CHUNKS
        idx_flat = np.zeros(total, np.int16)       # pad slots gather row 0
        idx_flat[slot] = (s_e - c_of * CHUNK).astype(np.int16)
        dv_flat = np.full(total, PAD_SENTINEL, np.float32)
        dv_flat[slot] = (dl & 127).astype(np.float32)
        # idx wrapped into 16 partitions, replicated x8 (one copy per Q7 core)
        idx_all.append(np.tile(idx_flat.reshape(total // 16, 16).T, (8, 1)).copy())
        dv_all.append(dv_flat.reshape(total // 128, 128).T.copy())
    return env, slot_off, total, idx_all, dv_all


def _raw_dma_gather(gp, out_ap, in_ap, idxs_ap, num_idxs, elem_size, elem_step):
    """dma_gather without the elem_size_bytes%256 assert (transpose-path-only
    restriction). Rows are elem_step-strided; each descriptor moves elem_size
    elements. Verified bit-exact on hardware for fp16 elem 64 / step 128."""
    import concourse.mybir as mybir
    import concourse.ap_utils as ap_utils
    assert in_ap.dtype == out_ap.dtype
    assert idxs_ap.dtype == mybir.dt.int16
    assert ap_utils.ap_is_contiguous(in_ap.ap[1:])
    assert ap_utils.ap_is_contiguous(out_ap.ap[1:])
    assert ap_utils.ap_is_contiguous(idxs_ap.ap[1:])
    assert in_ap.ap[-1][1] == out_ap.ap[-1][1] == elem_size
    assert in_ap.ap[0][0] == elem_step
    assert out_ap.ap[0][1] * out_ap.ap[1][1] == _round_up(num_idxs, 128)
    stride_bytes = elem_step * mybir.dt.size(in_ap.dtype)
    assert stride_bytes % 256 == 0 and stride_bytes // 256 < 256
    _in_ap = gp.lower_ap_dma(in_ap, for_custom_bir_dma=True)
    return gp.add_instruction(
        mybir.InstDMAGatherAnt(
            name=gp.bass.get_next_instruction_name(),
            ins=[*_in_ap, gp.lower_ap(idxs_ap),
                 gp.lower_val_access(gp.to_reg(num_idxs))],
            outs=[gp.lower_ap(out_ap)],
            transpose=False,
            num_idxs=num_idxs,
            elem_size=elem_size,
            stride_bytes_256=stride_bytes // 256,
            gen_mode=0,
            single_packet=False,
            queue_num=0,
        )
    )


def _build_program(env, slot_off, total):
    import concourse.bass as bass
    import concourse.bacc as bacc
    import concourse.mybir as mybir
    import concourse.tile as tile

    f32 = mybir.dt.float32
    fp16 = mybir.dt.float16
    n_blocks = total // 128
    seg_slots = env.sum(axis=2)                    # [S, C] slots per gather
    max_seg_blocks = int(seg_slots.max()) // 128

    nc = bacc.Bacc("TRN2", target_bir_lowering=False, debug=False,
                   num_devices=N_CORES)
    w_d = nc.dram_tensor("w", [N_NODES, 128], fp16, kind="ExternalInput").ap()
    ci_d = nc.dram_tensor("ci", [128, N_TILES], f32, kind="ExternalInput").ap()
    io_d = nc.dram_tensor("io", [128, 128], fp16, kind="ExternalInput").ap()
    idx_d = nc.dram_tensor("idx", [128, total // 16], mybir.dt.int16,
                           kind="ExternalInput").ap()
    dv_d = nc.dram_tensor("dv", [128, n_blocks], f32, kind="ExternalInput").ap()
    out_d = nc.dram_tensor("out", [128, N_TILES * OUT_DIM], f32,
                           kind="ExternalOutput").ap()

    with tile.TileContext(nc) as tc:
        with (
            tc.tile_pool(name="const", bufs=1) as constp,
            tc.tile_pool(name="msg", bufs=8) as msgp,
            tc.tile_pool(name="oh", bufs=32) as ohp,
            tc.tile_pool(name="ps", bufs=8, space="PSUM") as psp,
            tc.tile_pool(name="ot", bufs=2) as otp,
        ):
            ci_t = constp.tile([128, N_TILES], f32)
            io_t = constp.tile([128, 128], fp16)
            idx_t = constp.tile([128, total // 16], mybir.dt.int16)
            dv_t = constp.tile([128, n_blocks], f32)
            nc.sync.dma_start(ci_t[:], ci_d[:])
            nc.sync.dma_start(io_t[:], io_d[:])
            # chunk metadata uploads per supertile so early gathers/one-hots
            # only wait on their own slice
            for s in range(N_SUPER):
                a = int(slot_off[s * N_CHUNKS * G])
                b = int(slot_off[(s + 1) * N_CHUNKS * G]) if s + 1 < N_SUPER \
                    else total
                nc.sync.dma_start(idx_t[:, a // 16:b // 16],
                                  idx_d[:, a // 16:b // 16])
                nc.sync.dma_start(dv_t[:, a // 128:b // 128],
                                  dv_d[:, a // 128:b // 128])

            for s in range(N_SUPER):
                msgs = []
                for c in range(N_CHUNKS):
                    n_sc = int(seg_slots[s, c])
                    off = int(slot_off[(s * N_CHUNKS + c) * G])
                    msg = msgp.tile([128, max_seg_blocks, OUT_DIM], fp16,
                                    tag="msg")
                    _raw_dma_gather(
                        nc.gpsimd, msg[:, :n_sc // 128, :],
                        w_d[c * CHUNK:(c + 1) * CHUNK, 0:OUT_DIM],
                        idx_t[:, off // 16:(off + n_sc) // 16],
                        n_sc, OUT_DIM, 128)
                    msgs.append(msg)

                pss = [psp.tile([128, OUT_DIM], f32, tag="ps",
                                name=f"ps_{s}_{tl}")
                       for tl in range(G)]
                # per tile: count of blocks remaining (for start/stop flags)
                blk_total = [int(env[s, :, tl].sum()) // 128 for tl in range(G)]
                blk_seen = [0] * G
                for c in range(N_CHUNKS):
                    seg_base = int(slot_off[(s * N_CHUNKS + c) * G])
                    col = 0
                    for tl in range(G):
                        n_blk_t = int(env[s, c, tl]) // 128
                        for b in range(n_blk_t):
                            gcol = seg_base // 128 + col + b
                            oh = ohp.tile([128, 128], fp16, tag="oh")
                            nc.vector.tensor_scalar(
                                oh[:], io_t[:], dv_t[:, gcol:gcol + 1],
                                1.0, mybir.AluOpType.is_equal,
                                mybir.AluOpType.mult)
                            nc.tensor.matmul(
                                pss[tl][:], oh[:], msgs[c][:, col + b, :],
                                start=(blk_seen[tl] == 0),
                                stop=(blk_seen[tl] == blk_total[tl] - 1))
                            blk_seen[tl] += 1
                        col += n_blk_t

                ot = otp.tile([128, G * OUT_DIM], f32, tag="ot")
                for tl in range(G):
                    t = s * G + tl
                    nc.scalar.activation(
                        ot[:, tl * OUT_DIM:(tl + 1) * OUT_DIM], pss[tl][:],
                        mybir.ActivationFunctionType.Copy,
                        scale=ci_t[:, t:t + 1])
                nc.sync.dma_start(
                    out_d[:, s * G * OUT_DIM:(s + 1) * G * OUT_DIM], ot[:])

    nc.compile()
    return nc


def prepare(node_ids, src_idx, dst_idx, cj, ci, weight):
    """Host prep + program build. Returns (nc, in_maps, postprocess)."""
    import time
    _t0 = time.time()

    node_ids = np.asarray(node_ids)
    src = np.asarray(src_idx).astype(np.int64)
    dst = np.asarray(dst_idx).astype(np.int64)
    cj = np.asarray(cj, dtype=np.float32).reshape(-1)
    ci = np.asarray(ci, dtype=np.float32).reshape(-1)
    weight = np.asarray(weight, dtype=np.float32)

    # feat rows are weight[node_ids]; with the arange fill this is identity
    if not np.array_equal(node_ids, np.arange(N_NODES, dtype=node_ids.dtype)):
        weight = weight[node_ids]

    # prescale by cj and lay out as an fp16 table with 256B-strided rows
    w_tab = np.zeros((N_NODES, 128), np.float16)
    w_tab[:, :OUT_DIM] = (weight * cj[:, None]).astype(np.float16)

    iota = np.tile(np.arange(128, dtype=np.float16), (128, 1))

    env, slot_off, total, idx_all, dv_all = _host_prep(src, dst)
    print(f"[kernel] host prep: {time.time()-_t0:.1f}s (total slots {total})",
          flush=True)
    _t1 = time.time()
    nc = _build_program(env, slot_off, total)
    print(f"[kernel] build+schedule+compile-to-bir: {time.time()-_t1:.1f}s",
          flush=True)

    in_maps = []
    for k in range(N_CORES):
        ci_k = np.zeros(N_TILES * 128, np.float32)
        ci_k[:DST_PER_CORE] = ci[k * DST_PER_CORE:(k + 1) * DST_PER_CORE]
        ci_w = ci_k.reshape(N_TILES, 128).T.copy()
        in_maps.append({
            "w": w_tab, "ci": ci_w, "io": iota,
            "idx": idx_all[k], "dv": dv_all[k],
        })

    def post(results):
        outs = []
        for k in range(N_CORES):
            o = np.asarray(results[k]["out"])        # [128, 98*64]
            o = o.reshape(128, N_TILES, OUT_DIM).transpose(1, 0, 2)
            outs.append(o.reshape(-1, OUT_DIM)[:DST_PER_CORE])
        return np.concatenate(outs, axis=0)

    return nc, in_maps, post


def kernel(node_ids, src_idx, dst_idx, cj, ci, weight):
    import time
    from concourse.bass_utils import run_bass_kernel_spmd
    nc, in_maps, post = prepare(node_ids, src_idx, dst_idx, cj, ci, weight)
    _t2 = time.time()
    res = run_bass_kernel_spmd(nc, in_maps, core_ids=list(range(N_CORES)))
    print(f"[kernel] neff compile+exec: {time.time()-_t2:.1f}s", flush=True)
    return post(res.results)


# revision 7
# speedup vs baseline: 1.1116x; 1.0173x over previous
"""GCMC graph-conv kernel for Trainium2, 8-core SPMD.

out = ci * segment_sum((weight[node_ids] * cj)[src_idx], dst_idx)

Strategy (edge sharding by dst range, fp16 message path):
  - host prescales W' = weight[node_ids] * cj, stores it as an fp16 table with
    256B-strided rows ([100000, 128] fp16, data in cols 0:64) so each gather
    descriptor moves only 128B (half the DMA time of a 256B fp32 row)
  - core k owns dst rows [k*12500, (k+1)*12500); its edges are partitioned by
    (supertile of G=4 dst tiles, src chunk of 25000, dst tile) with each
    (supertile, chunk, tile) sub-segment padded to 128 slots using a shared
    static envelope (max over cores) so the program is SPMD-identical
  - one SWDGE dma_gather per (supertile, chunk) — 100 gathers instead of 392 —
    emitted raw (the bass wrapper's elem%256 assert is a transpose-path
    restriction; elem_step=128/elem_size=64 fp16 is valid and verified on hw)
  - segment-sum via one-hot matmul on DVE+PE: oh[slot, d] = (iota[d] ==
    dv[slot]); psum[dst, 64] += oh.T @ msg in fp16; psum groups are
    double-buffered (2 supertiles x 4 banks) so one supertile's tail overlaps
    the next's head
  - idx/dv metadata staged through rotating pools inside the supertile loop so
    uploads interleave with gathers instead of front-loading the DMA queue
  - flush: ACT copies psum*ci into a per-supertile staging tile, one HWDGE
    DMA per supertile writes [128, G*64] fp32 to a partition-major output
    buffer that the host untransposes
"""
import sys, os
sys.path.insert(0, '/opt/trn_rl_repo')

import numpy as np

N_NODES = 100000
OUT_DIM = 64
N_CORES = 8
DST_PER_CORE = N_NODES // N_CORES          # 12500
N_TILES = (DST_PER_CORE + 127) // 128      # 98
G = 4                                      # dst tiles per supertile
N_SUPER = (N_TILES + G - 1) // G           # 25 (last has 2 tiles)
G_OF = [min(G, N_TILES - s * G) for s in range(N_SUPER)]
N_CHUNKS = 4                               # int16 idx -> <=25000 rows per chunk
CHUNK = N_NODES // N_CHUNKS                # 25000
PAD_SENTINEL = 999.0


def _round_up(x, m):
    return (x + m - 1) // m * m


def _host_prep(src, dst):
    """Partition edges by dst core range; compute the shared static envelope
    env[t, c] (max per-core (tile, chunk) count, rounded to 128) and per-core
    slot-packed idx / dv arrays laid out in envelope slots ordered by
    (supertile, chunk, tile)."""
    per_core = []
    counts = np.zeros((N_CORES, N_TILES, N_CHUNKS), np.int64)
    for k in range(N_CORES):
        m = (dst // DST_PER_CORE) == k
        s_e = src[m]
        dl = dst[m] - k * DST_PER_CORE
        t = dl >> 7                         # dst tile 0..97
        c = s_e // CHUNK                    # src chunk 0..3
        counts[k] = np.bincount(t * N_CHUNKS + c,
                                minlength=N_TILES * N_CHUNKS).reshape(
                                    N_TILES, N_CHUNKS)
        per_core.append((s_e, dl, t, c))

    env = _round_up(counts.max(axis=0), 128)       # [T, C]
    # slot order: (supertile, chunk, tile)
    grp_key = []
    for s in range(N_SUPER):
        for c in range(N_CHUNKS):
            for tl in range(G_OF[s]):
                grp_key.append((s * G + tl, c))
    env_seq = np.array([env[t, c] for (t, c) in grp_key], np.int64)
    slot_off_seq = np.concatenate([[0], np.cumsum(env_seq)])[:-1]
    total = int(env_seq.sum())
    slot_off = np.zeros((N_TILES, N_CHUNKS), np.int64)
    for g, (t, c) in enumerate(grp_key):
        slot_off[t, c] = slot_off_seq[g]

    idx_all, dv_all = [], []
    for k in range(N_CORES):
        s_e, dl, t, c = per_core[k]
        gid = t * N_CHUNKS + c
        order = np.argsort(gid, kind='stable')
        s_e, dl, t, c, gid = (s_e[order], dl[order], t[order], c[order],
                              gid[order])
        gcounts = np.bincount(gid, minlength=N_TILES * N_CHUNKS)
        within = np.arange(len(s_e)) - np.repeat(
            np.concatenate([[0], np.cumsum(gcounts)])[:-1], gcounts)
        slot = slot_off[t, c] + within
        idx_flat = np.zeros(total, np.int16)       # pad slots gather row 0
        idx_flat[slot] = (s_e - c * CHUNK).astype(np.int16)
        dv_flat = np.full(total, PAD_SENTINEL, np.float32)
        dv_flat[slot] = (dl & 127).astype(np.float32)
        # idx wrapped into 16 partitions, replicated x8 (one copy per Q7 core)
        idx_all.append(np.tile(idx_flat.reshape(total // 16, 16).T, (8, 1)).copy())
        dv_all.append(dv_flat.reshape(total // 128, 128).T.copy())
    return env, slot_off, total, idx_all, dv_all


def _raw_dma_gather(gp, out_ap, in_ap, idxs_ap, num_idxs, elem_size, elem_step):
    """dma_gather without the elem_size_bytes%256 assert (transpose-path-only
    restriction). Rows are elem_step-strided; each descriptor moves elem_size
    elements. Verified bit-exact on hardware for fp16 elem 64 / step 128."""
    import concourse.mybir as mybir
    import concourse.ap_utils as ap_utils
    assert in_ap.dtype == out_ap.dtype
    assert idxs_ap.dtype == mybir.dt.int16
    assert ap_utils.ap_is_contiguous(in_ap.ap[1:])
    assert ap_utils.ap_is_contiguous(out_ap.ap[1:])
    assert ap_utils.ap_is_contiguous(idxs_ap.ap[1:])
    assert in_ap.ap[-1][1] == out_ap.ap[-1][1] == elem_size
    assert in_ap.ap[0][0] == elem_step
    assert out_ap.ap[0][1] * out_ap.ap[1][1] == _round_up(num_idxs, 128)
    stride_bytes = elem_step * mybir.dt.size(in_ap.dtype)
    assert stride_bytes % 256 == 0 and stride_bytes // 256 < 256
    _in_ap = gp.lower_ap_dma(in_ap, for_custom_bir_dma=True)
    return gp.add_instruction(
        mybir.InstDMAGatherAnt(
            name=gp.bass.get_next_instruction_name(),
            ins=[*_in_ap, gp.lower_ap(idxs_ap),
                 gp.lower_val_access(gp.to_reg(num_idxs))],
            outs=[gp.lower_ap(out_ap)],
            transpose=False,
            num_idxs=num_idxs,
            elem_size=elem_size,
            stride_bytes_256=stride_bytes // 256,
            gen_mode=0,
            single_packet=False,
            queue_num=0,
        )
    )


def _build_program(env, slot_off, total):
    import concourse.bass as bass
    import concourse.bacc as bacc
    import concourse.mybir as mybir
    import concourse.tile as tile

    f32 = mybir.dt.float32
    fp16 = mybir.dt.float16

    sup_slots = []
    for s in range(N_SUPER):
        n = sum(int(env[s * G + tl, c])
                for c in range(N_CHUNKS) for tl in range(G_OF[s]))
        sup_slots.append(n)
    sup_off = np.concatenate([[0], np.cumsum(sup_slots)])[:-1].astype(np.int64)
    max_sup_slots = _round_up(max(sup_slots), 2048)
    max_seg_blocks = max(
        sum(int(env[s * G + tl, c]) for tl in range(G_OF[s])) // 128
        for s in range(N_SUPER) for c in range(N_CHUNKS))

    nc = bacc.Bacc("TRN2", target_bir_lowering=False, debug=False,
                   num_devices=N_CORES)
    w_d = nc.dram_tensor("w", [N_NODES, 128], fp16, kind="ExternalInput").ap()
    ci_d = nc.dram_tensor("ci", [128, N_TILES], f32, kind="ExternalInput").ap()
    io_d = nc.dram_tensor("io", [128, 128], fp16, kind="ExternalInput").ap()
    idx_d = nc.dram_tensor("idx", [128, total // 16], mybir.dt.int16,
                           kind="ExternalInput").ap()
    dv_d = nc.dram_tensor("dv", [128, total // 128], f32,
                          kind="ExternalInput").ap()
    out_d = nc.dram_tensor("out", [128, N_TILES * OUT_DIM], f32,
                           kind="ExternalOutput").ap()

    with tile.TileContext(nc) as tc:
        with (
            tc.tile_pool(name="const", bufs=1) as constp,
            tc.tile_pool(name="idxp", bufs=3) as idxp,
            tc.tile_pool(name="dvp", bufs=3) as dvp,
            tc.tile_pool(name="msg", bufs=10) as msgp,
            tc.tile_pool(name="oh", bufs=32) as ohp,
            tc.tile_pool(name="ps", bufs=8, space="PSUM") as psp,
            tc.tile_pool(name="ot", bufs=3) as otp,
        ):
            ci_t = constp.tile([128, N_TILES], f32)
            io_t = constp.tile([128, 128], fp16)
            nc.sync.dma_start(ci_t[:], ci_d[:])
            nc.sync.dma_start(io_t[:], io_d[:])

            for s in range(N_SUPER):
                gs = G_OF[s]
                base = int(sup_off[s])
                nsl = sup_slots[s]
                # stage this supertile's metadata (rotating pools provide
                # back-pressure so uploads interleave with earlier gathers)
                idx_t = idxp.tile([128, max_sup_slots // 16], mybir.dt.int16,
                                  tag="idx", name=f"idx_{s}")
                dv_t = dvp.tile([128, max_sup_slots // 128], f32,
                                tag="dv", name=f"dv_{s}")
                nc.sync.dma_start(idx_t[:, :nsl // 16],
                                  idx_d[:, base // 16:(base + nsl) // 16])
                nc.sync.dma_start(dv_t[:, :nsl // 128],
                                  dv_d[:, base // 128:(base + nsl) // 128])

                msgs = []
                rel = 0                              # slot offset within super
                seg_rel = []
                for c in range(N_CHUNKS):
                    n_sc = sum(int(env[s * G + tl, c]) for tl in range(gs))
                    msg = msgp.tile([128, max_seg_blocks, OUT_DIM], fp16,
                                    tag="msg", name=f"msg_{s}_{c}")
                    _raw_dma_gather(
                        nc.gpsimd, msg[:, :n_sc // 128, :],
                        w_d[c * CHUNK:(c + 1) * CHUNK, 0:OUT_DIM],
                        idx_t[:, rel // 16:(rel + n_sc) // 16],
                        n_sc, OUT_DIM, 128)
                    msgs.append(msg)
                    seg_rel.append(rel)
                    rel += n_sc

                pss = [psp.tile([128, OUT_DIM], f32, tag="ps",
                                name=f"ps_{s}_{tl}") for tl in range(gs)]
                blk_total = [sum(int(env[s * G + tl, c])
                                 for c in range(N_CHUNKS)) // 128
                             for tl in range(gs)]
                blk_seen = [0] * gs
                for c in range(N_CHUNKS):
                    col = 0
                    for tl in range(gs):
                        n_blk_t = int(env[s * G + tl, c]) // 128
                        for b in range(n_blk_t):
                            gcol = (seg_rel[c] + 128 * (col + b)) // 128
                            oh = ohp.tile([128, 128], fp16, tag="oh")
                            nc.vector.tensor_scalar(
                                oh[:], io_t[:], dv_t[:, gcol:gcol + 1],
                                1.0, mybir.AluOpType.is_equal,
                                mybir.AluOpType.mult)
                            nc.tensor.matmul(
                                pss[tl][:], oh[:], msgs[c][:, col + b, :],
                                start=(blk_seen[tl] == 0),
                                stop=(blk_seen[tl] == blk_total[tl] - 1))
                            blk_seen[tl] += 1
                        col += n_blk_t

                ot = otp.tile([128, gs * OUT_DIM], f32, tag="ot",
                              name=f"ot_{s}")
                for tl in range(gs):
                    t = s * G + tl
                    nc.scalar.activation(
                        ot[:, tl * OUT_DIM:(tl + 1) * OUT_DIM], pss[tl][:],
                        mybir.ActivationFunctionType.Copy,
                        scale=ci_t[:, t:t + 1])
                nc.sync.dma_start(
                    out_d[:, s * G * OUT_DIM:(s * G + gs) * OUT_DIM], ot[:])

    nc.compile()
    return nc


def prepare(node_ids, src_idx, dst_idx, cj, ci, weight):
    """Host prep + program build. Returns (nc, in_maps, postprocess)."""
    import time
    _t0 = time.time()

    node_ids = np.asarray(node_ids)
    src = np.asarray(src_idx).astype(np.int64)
    dst = np.asarray(dst_idx).astype(np.int64)
    cj = np.asarray(cj, dtype=np.float32).reshape(-1)
    ci = np.asarray(ci, dtype=np.float32).reshape(-1)
    weight = np.asarray(weight, dtype=np.float32)

    # feat rows are weight[node_ids]; with the arange fill this is identity
    if not np.array_equal(node_ids, np.arange(N_NODES, dtype=node_ids.dtype)):
        weight = weight[node_ids]

    # prescale by cj and lay out as an fp16 table with 256B-strided rows
    w_tab = np.zeros((N_NODES, 128), np.float16)
    w_tab[:, :OUT_DIM] = (weight * cj[:, None]).astype(np.float16)

    iota = np.tile(np.arange(128, dtype=np.float16), (128, 1))

    env, slot_off, total, idx_all, dv_all = _host_prep(src, dst)
    print(f"[kernel] host prep: {time.time()-_t0:.1f}s (total slots {total})",
          flush=True)
    _t1 = time.time()
    nc = _build_program(env, slot_off, total)
    print(f"[kernel] build+schedule+compile-to-bir: {time.time()-_t1:.1f}s",
          flush=True)

    in_maps = []
    for k in range(N_CORES):
        ci_k = np.zeros(N_TILES * 128, np.float32)
        ci_k[:DST_PER_CORE] = ci[k * DST_PER_CORE:(k + 1) * DST_PER_CORE]
        ci_w = ci_k.reshape(N_TILES, 128).T.copy()
        in_maps.append({
            "w": w_tab, "ci": ci_w, "io": iota,
            "idx": idx_all[k], "dv": dv_all[k],
        })

    def post(results):
        outs = []
        for k in range(N_CORES):
            o = np.asarray(results[k]["out"])        # [128, 98*64]
            o = o.reshape(128, N_TILES, OUT_DIM).transpose(1, 0, 2)
            outs.append(o.reshape(-1, OUT_DIM)[:DST_PER_CORE])
        return np.concatenate(outs, axis=0)

    return nc, in_maps, post


def kernel(node_ids, src_idx, dst_idx, cj, ci, weight):
    import time
    from concourse.bass_utils import run_bass_kernel_spmd
    nc, in_maps, post = prepare(node_ids, src_idx, dst_idx, cj, ci, weight)
    _t2 = time.time()
    res = run_bass_kernel_spmd(nc, in_maps, core_ids=list(range(N_CORES)))
    print(f"[kernel] neff compile+exec: {time.time()-_t2:.1f}s", flush=True)
    return post(res.results)


# revision 8
# speedup vs baseline: 1.5307x; 1.3770x over previous
"""GCMC graph-conv kernel for Trainium2, 8-core SPMD.

out = ci * segment_sum((weight[node_ids] * cj)[src_idx], dst_idx)

Strategy (edge sharding by dst range, fp16 message path):
  - host prescales W' = weight[node_ids] * cj, stores it as an fp16 table with
    256B-strided rows ([100000, 128] fp16, data in cols 0:64) so each gather
    descriptor moves only 128B (half the DMA time of a 256B fp32 row)
  - core k owns dst rows [k*12500, (k+1)*12500); its edges are partitioned by
    (supertile of G=4 dst tiles, src chunk of 25000, dst tile) with each
    (supertile, chunk, tile) sub-segment padded to 128 slots using a shared
    static envelope (max over cores) so the program is SPMD-identical
  - one SWDGE dma_gather per (supertile, chunk) — 100 gathers instead of 392 —
    emitted raw (the bass wrapper's elem%256 assert is a transpose-path
    restriction; elem_step=128/elem_size=64 fp16 is valid and verified on hw)
  - segment-sum via one-hot matmul on DVE+PE: oh[slot, d] = (iota[d] ==
    dv[slot]); psum[dst, 64] += oh.T @ msg in fp16; psum groups are
    double-buffered (2 supertiles x 4 banks) so one supertile's tail overlaps
    the next's head
  - idx/dv metadata staged through rotating pools inside the supertile loop so
    uploads interleave with gathers instead of front-loading the DMA queue
  - flush: ACT copies psum*ci into a per-supertile staging tile, one HWDGE
    DMA per supertile writes [128, G*64] fp32 to a partition-major output
    buffer that the host untransposes
"""
import sys, os
sys.path.insert(0, '/opt/trn_rl_repo')

import numpy as np

N_NODES = 100000
OUT_DIM = 64
N_CORES = 8
DST_PER_CORE = N_NODES // N_CORES          # 12500
N_TILES = (DST_PER_CORE + 127) // 128      # 98
G = 4                                      # dst tiles per supertile
N_SUPER = (N_TILES + G - 1) // G           # 25 (last has 2 tiles)
G_OF = [min(G, N_TILES - s * G) for s in range(N_SUPER)]
N_CHUNKS = 4                               # int16 idx -> <=25000 rows per chunk
CHUNK = N_NODES // N_CHUNKS                # 25000
PAD_SENTINEL = 999.0


def _round_up(x, m):
    return (x + m - 1) // m * m


def _host_prep(src, dst):
    """Partition edges by dst core range; compute the shared static envelope
    env[t, c] (max per-core (tile, chunk) count, rounded to 128) and per-core
    slot-packed idx / dv arrays laid out in envelope slots ordered by
    (supertile, chunk, tile)."""
    per_core = []
    counts = np.zeros((N_CORES, N_TILES, N_CHUNKS), np.int64)
    for k in range(N_CORES):
        m = (dst // DST_PER_CORE) == k
        s_e = src[m]
        dl = dst[m] - k * DST_PER_CORE
        t = dl >> 7                         # dst tile 0..97
        c = s_e // CHUNK                    # src chunk 0..3
        counts[k] = np.bincount(t * N_CHUNKS + c,
                                minlength=N_TILES * N_CHUNKS).reshape(
                                    N_TILES, N_CHUNKS)
        per_core.append((s_e, dl, t, c))

    env = _round_up(counts.max(axis=0), 128)       # [T, C]
    # slot order: (supertile, chunk, tile)
    grp_key = []
    for s in range(N_SUPER):
        for c in range(N_CHUNKS):
            for tl in range(G_OF[s]):
                grp_key.append((s * G + tl, c))
    env_seq = np.array([env[t, c] for (t, c) in grp_key], np.int64)
    slot_off_seq = np.concatenate([[0], np.cumsum(env_seq)])[:-1]
    total = int(env_seq.sum())
    slot_off = np.zeros((N_TILES, N_CHUNKS), np.int64)
    for g, (t, c) in enumerate(grp_key):
        slot_off[t, c] = slot_off_seq[g]

    idx_all, dv_all = [], []
    for k in range(N_CORES):
        s_e, dl, t, c = per_core[k]
        gid = t * N_CHUNKS + c
        order = np.argsort(gid, kind='stable')
        s_e, dl, t, c, gid = (s_e[order], dl[order], t[order], c[order],
                              gid[order])
        gcounts = np.bincount(gid, minlength=N_TILES * N_CHUNKS)
        within = np.arange(len(s_e)) - np.repeat(
            np.concatenate([[0], np.cumsum(gcounts)])[:-1], gcounts)
        slot = slot_off[t, c] + within
        idx_flat = np.zeros(total, np.int16)       # pad slots gather row 0
        idx_flat[slot] = (s_e - c * CHUNK).astype(np.int16)
        dv_flat = np.full(total, PAD_SENTINEL, np.float32)
        dv_flat[slot] = (dl & 127).astype(np.float32)
        # idx wrapped into 16 partitions, replicated x8 (one copy per Q7 core)
        idx_all.append(np.tile(idx_flat.reshape(total // 16, 16).T, (8, 1)).copy())
        dv_all.append(dv_flat.reshape(total // 128, 128).T.copy())
    return env, slot_off, total, idx_all, dv_all


def _raw_dma_gather(gp, out_ap, in_ap, idxs_ap, num_idxs, elem_size, elem_step):
    """dma_gather without the elem_size_bytes%256 assert (transpose-path-only
    restriction). Rows are elem_step-strided; each descriptor moves elem_size
    elements. Verified bit-exact on hardware for fp16 elem 64 / step 128."""
    import concourse.mybir as mybir
    import concourse.ap_utils as ap_utils
    assert in_ap.dtype == out_ap.dtype
    assert idxs_ap.dtype == mybir.dt.int16
    assert ap_utils.ap_is_contiguous(in_ap.ap[1:])
    assert ap_utils.ap_is_contiguous(out_ap.ap[1:])
    assert ap_utils.ap_is_contiguous(idxs_ap.ap[1:])
    assert in_ap.ap[-1][1] == out_ap.ap[-1][1] == elem_size
    assert in_ap.ap[0][0] == elem_step
    assert out_ap.ap[0][1] * out_ap.ap[1][1] == _round_up(num_idxs, 128)
    stride_bytes = elem_step * mybir.dt.size(in_ap.dtype)
    assert stride_bytes % 256 == 0 and stride_bytes // 256 < 256
    _in_ap = gp.lower_ap_dma(in_ap, for_custom_bir_dma=True)
    return gp.add_instruction(
        mybir.InstDMAGatherAnt(
            name=gp.bass.get_next_instruction_name(),
            ins=[*_in_ap, gp.lower_ap(idxs_ap),
                 gp.lower_val_access(gp.to_reg(num_idxs))],
            outs=[gp.lower_ap(out_ap)],
            transpose=False,
            num_idxs=num_idxs,
            elem_size=elem_size,
            stride_bytes_256=stride_bytes // 256,
            gen_mode=0,
            single_packet=False,
            queue_num=0,
        )
    )


def _build_program(env, slot_off, total):
    import concourse.bass as bass
    import concourse.bacc as bacc
    import concourse.mybir as mybir
    import concourse.tile as tile

    f32 = mybir.dt.float32
    fp16 = mybir.dt.float16

    sup_slots = []
    for s in range(N_SUPER):
        n = sum(int(env[s * G + tl, c])
                for c in range(N_CHUNKS) for tl in range(G_OF[s]))
        sup_slots.append(n)
    sup_off = np.concatenate([[0], np.cumsum(sup_slots)])[:-1].astype(np.int64)
    max_sup_slots = _round_up(max(sup_slots), 2048)
    max_seg_blocks = max(
        sum(int(env[s * G + tl, c]) for tl in range(G_OF[s])) // 128
        for s in range(N_SUPER) for c in range(N_CHUNKS))

    nc = bacc.Bacc("TRN2", target_bir_lowering=False, debug=False,
                   num_devices=N_CORES)
    w_d = nc.dram_tensor("w", [N_NODES, 128], fp16, kind="ExternalInput").ap()
    ci_d = nc.dram_tensor("ci", [128, N_TILES], f32, kind="ExternalInput").ap()
    io_d = nc.dram_tensor("io", [128, 128], fp16, kind="ExternalInput").ap()
    idx_d = nc.dram_tensor("idx", [128, total // 16], mybir.dt.int16,
                           kind="ExternalInput").ap()
    dv_d = nc.dram_tensor("dv", [128, total // 128], f32,
                          kind="ExternalInput").ap()
    out_d = nc.dram_tensor("out", [128, N_TILES * OUT_DIM], f32,
                           kind="ExternalOutput").ap()

    with tile.TileContext(nc) as tc:
        with (
            tc.tile_pool(name="const", bufs=1) as constp,
            tc.tile_pool(name="idxp", bufs=3) as idxp,
            tc.tile_pool(name="dvp", bufs=3) as dvp,
            tc.tile_pool(name="msg", bufs=10) as msgp,
            tc.tile_pool(name="oh", bufs=2) as ohp,
            tc.tile_pool(name="ps", bufs=8, space="PSUM") as psp,
            tc.tile_pool(name="ot", bufs=3) as otp,
        ):
            ci_t = constp.tile([128, N_TILES], f32)
            io_t = constp.tile([128, 128], fp16)
            nc.sync.dma_start(ci_t[:], ci_d[:])
            nc.sync.dma_start(io_t[:], io_d[:])

            for s in range(N_SUPER):
                gs = G_OF[s]
                base = int(sup_off[s])
                nsl = sup_slots[s]
                # stage this supertile's metadata (rotating pools provide
                # back-pressure so uploads interleave with earlier gathers)
                idx_t = idxp.tile([128, max_sup_slots // 16], mybir.dt.int16,
                                  tag="idx", name=f"idx_{s}")
                dv_t = dvp.tile([128, max_sup_slots // 128], f32,
                                tag="dv", name=f"dv_{s}")
                nc.sync.dma_start(idx_t[:, :nsl // 16],
                                  idx_d[:, base // 16:(base + nsl) // 16])
                nc.sync.dma_start(dv_t[:, :nsl // 128],
                                  dv_d[:, base // 128:(base + nsl) // 128])

                msgs = []
                rel = 0                              # slot offset within super
                seg_rel = []
                for c in range(N_CHUNKS):
                    n_sc = sum(int(env[s * G + tl, c]) for tl in range(gs))
                    msg = msgp.tile([128, max_seg_blocks, OUT_DIM], fp16,
                                    tag="msg", name=f"msg_{s}_{c}")
                    _raw_dma_gather(
                        nc.gpsimd, msg[:, :n_sc // 128, :],
                        w_d[c * CHUNK:(c + 1) * CHUNK, 0:OUT_DIM],
                        idx_t[:, rel // 16:(rel + n_sc) // 16],
                        n_sc, OUT_DIM, 128)
                    msgs.append(msg)
                    seg_rel.append(rel)
                    rel += n_sc

                n_blk_sup = nsl // 128
                oh_sup = ohp.tile([128, n_blk_sup, 128], fp16, tag="oh",
                                  name=f"oh_{s}", padded_shape=None)
                pss = [psp.tile([128, OUT_DIM], f32, tag="ps",
                                name=f"ps_{s}_{tl}") for tl in range(gs)]
                blk_total = [sum(int(env[s * G + tl, c])
                                 for c in range(N_CHUNKS)) // 128
                             for tl in range(gs)]
                blk_seen = [0] * gs
                for c in range(N_CHUNKS):
                    col = 0
                    for tl in range(gs):
                        n_blk_t = int(env[s * G + tl, c]) // 128
                        for b in range(n_blk_t):
                            gcol = (seg_rel[c] + 128 * (col + b)) // 128
                            nc.vector.tensor_scalar(
                                oh_sup[:, gcol, :], io_t[:],
                                dv_t[:, gcol:gcol + 1],
                                1.0, mybir.AluOpType.is_equal,
                                mybir.AluOpType.mult)
                            nc.tensor.matmul(
                                pss[tl][:], oh_sup[:, gcol, :],
                                msgs[c][:, col + b, :],
                                start=(blk_seen[tl] == 0),
                                stop=(blk_seen[tl] == blk_total[tl] - 1))
                            blk_seen[tl] += 1
                        col += n_blk_t

                ot = otp.tile([128, gs * OUT_DIM], f32, tag="ot",
                              name=f"ot_{s}")
                for tl in range(gs):
                    t = s * G + tl
                    nc.scalar.activation(
                        ot[:, tl * OUT_DIM:(tl + 1) * OUT_DIM], pss[tl][:],
                        mybir.ActivationFunctionType.Copy,
                        scale=ci_t[:, t:t + 1])
                nc.sync.dma_start(
                    out_d[:, s * G * OUT_DIM:(s * G + gs) * OUT_DIM], ot[:])

    nc.compile()
    return nc


def prepare(node_ids, src_idx, dst_idx, cj, ci, weight):
    """Host prep + program build. Returns (nc, in_maps, postprocess)."""
    import time
    _t0 = time.time()

    node_ids = np.asarray(node_ids)
    src = np.asarray(src_idx).astype(np.int64)
    dst = np.asarray(dst_idx).astype(np.int64)
    cj = np.asarray(cj, dtype=np.float32).reshape(-1)
    ci = np.asarray(ci, dtype=np.float32).reshape(-1)
    weight = np.asarray(weight, dtype=np.float32)

    # feat rows are weight[node_ids]; with the arange fill this is identity
    if not np.array_equal(node_ids, np.arange(N_NODES, dtype=node_ids.dtype)):
        weight = weight[node_ids]

    # prescale by cj and lay out as an fp16 table with 256B-strided rows
    w_tab = np.zeros((N_NODES, 128), np.float16)
    w_tab[:, :OUT_DIM] = (weight * cj[:, None]).astype(np.float16)

    iota = np.tile(np.arange(128, dtype=np.float16), (128, 1))

    env, slot_off, total, idx_all, dv_all = _host_prep(src, dst)
    print(f"[kernel] host prep: {time.time()-_t0:.1f}s (total slots {total})",
          flush=True)
    _t1 = time.time()
    nc = _build_program(env, slot_off, total)
    print(f"[kernel] build+schedule+compile-to-bir: {time.time()-_t1:.1f}s",
          flush=True)

    in_maps = []
    for k in range(N_CORES):
        ci_k = np.zeros(N_TILES * 128, np.float32)
        ci_k[:DST_PER_CORE] = ci[k * DST_PER_CORE:(k + 1) * DST_PER_CORE]
        ci_w = ci_k.reshape(N_TILES, 128).T.copy()
        in_maps.append({
            "w": w_tab, "ci": ci_w, "io": iota,
            "idx": idx_all[k], "dv": dv_all[k],
        })

    def post(results):
        outs = []
        for k in range(N_CORES):
            o = np.asarray(results[k]["out"])        # [128, 98*64]
            o = o.reshape(128, N_TILES, OUT_DIM).transpose(1, 0, 2)
            outs.append(o.reshape(-1, OUT_DIM)[:DST_PER_CORE])
        return np.concatenate(outs, axis=0)

    return nc, in_maps, post


def kernel(node_ids, src_idx, dst_idx, cj, ci, weight):
    import time
    from concourse.bass_utils import run_bass_kernel_spmd
    nc, in_maps, post = prepare(node_ids, src_idx, dst_idx, cj, ci, weight)
    _t2 = time.time()
    res = run_bass_kernel_spmd(nc, in_maps, core_ids=list(range(N_CORES)))
    print(f"[kernel] neff compile+exec: {time.time()-_t2:.1f}s", flush=True)
    return post(res.results)


# revision 9
# speedup vs baseline: 1.5499x; 1.0126x over previous
"""GCMC graph-conv kernel for Trainium2, 8-core SPMD.

out = ci * segment_sum((weight[node_ids] * cj)[src_idx], dst_idx)

Strategy (edge sharding by dst range, fp16 message path):
  - host prescales W' = weight[node_ids] * cj, stores it as an fp16 table with
    256B-strided rows ([100000, 128] fp16, data in cols 0:64) so each gather
    descriptor moves only 128B (half the DMA time of a 256B fp32 row)
  - core k owns dst rows [k*12500, (k+1)*12500); its edges are partitioned by
    (supertile of G=4 dst tiles, src chunk of 25000, dst tile) with each
    (supertile, chunk, tile) sub-segment padded to 128 slots using a shared
    static envelope (max over cores) so the program is SPMD-identical
  - one SWDGE dma_gather per (supertile, chunk) — 100 gathers instead of 392 —
    emitted raw (the bass wrapper's elem%256 assert is a transpose-path
    restriction; elem_step=128/elem_size=64 fp16 is valid and verified on hw)
  - segment-sum via one-hot matmul on DVE+PE: oh[slot, d] = (iota[d] ==
    dv[slot]); psum[dst, 64] += oh.T @ msg in fp16; psum groups are
    double-buffered (2 supertiles x 4 banks) so one supertile's tail overlaps
    the next's head
  - idx/dv metadata staged through rotating pools inside the supertile loop so
    uploads interleave with gathers instead of front-loading the DMA queue
  - flush: ACT copies psum*ci into a per-supertile staging tile, one HWDGE
    DMA per supertile writes [128, G*64] fp32 to a partition-major output
    buffer that the host untransposes
"""
import sys, os
sys.path.insert(0, '/opt/trn_rl_repo')

import numpy as np

N_NODES = 100000
OUT_DIM = 64
N_CORES = 8
DST_PER_CORE = N_NODES // N_CORES          # 12500
N_TILES = (DST_PER_CORE + 127) // 128      # 98
G = 4                                      # dst tiles per supertile
N_SUPER = (N_TILES + G - 1) // G           # 25 (last has 2 tiles)
G_OF = [min(G, N_TILES - s * G) for s in range(N_SUPER)]
N_CHUNKS = 4                               # int16 idx -> <=25000 rows per chunk
CHUNK = N_NODES // N_CHUNKS                # 25000
PAD_SENTINEL = 999.0


def _round_up(x, m):
    return (x + m - 1) // m * m


def _host_prep(src, dst):
    """Partition edges by dst core range; compute the shared static envelope
    env[t, c] (max per-core (tile, chunk) count, rounded to 128) and per-core
    slot-packed idx / dv arrays laid out in envelope slots ordered by
    (supertile, chunk, tile)."""
    per_core = []
    counts = np.zeros((N_CORES, N_TILES, N_CHUNKS), np.int64)
    for k in range(N_CORES):
        m = (dst // DST_PER_CORE) == k
        s_e = src[m]
        dl = dst[m] - k * DST_PER_CORE
        t = dl >> 7                         # dst tile 0..97
        c = s_e // CHUNK                    # src chunk 0..3
        counts[k] = np.bincount(t * N_CHUNKS + c,
                                minlength=N_TILES * N_CHUNKS).reshape(
                                    N_TILES, N_CHUNKS)
        per_core.append((s_e, dl, t, c))

    env = _round_up(counts.max(axis=0), 128)       # [T, C]
    # slot order: (supertile, chunk, tile)
    grp_key = []
    for s in range(N_SUPER):
        for c in range(N_CHUNKS):
            for tl in range(G_OF[s]):
                grp_key.append((s * G + tl, c))
    env_seq = np.array([env[t, c] for (t, c) in grp_key], np.int64)
    slot_off_seq = np.concatenate([[0], np.cumsum(env_seq)])[:-1]
    total = int(env_seq.sum())
    slot_off = np.zeros((N_TILES, N_CHUNKS), np.int64)
    for g, (t, c) in enumerate(grp_key):
        slot_off[t, c] = slot_off_seq[g]

    idx_all, dv_all = [], []
    for k in range(N_CORES):
        s_e, dl, t, c = per_core[k]
        gid = t * N_CHUNKS + c
        order = np.argsort(gid, kind='stable')
        s_e, dl, t, c, gid = (s_e[order], dl[order], t[order], c[order],
                              gid[order])
        gcounts = np.bincount(gid, minlength=N_TILES * N_CHUNKS)
        within = np.arange(len(s_e)) - np.repeat(
            np.concatenate([[0], np.cumsum(gcounts)])[:-1], gcounts)
        slot = slot_off[t, c] + within
        idx_flat = np.zeros(total, np.int16)       # pad slots gather row 0
        idx_flat[slot] = (s_e - c * CHUNK).astype(np.int16)
        dv_flat = np.full(total, PAD_SENTINEL, np.float32)
        dv_flat[slot] = (dl & 127).astype(np.float32)
        # idx wrapped into 16 partitions, replicated x8 (one copy per Q7 core)
        idx_all.append(np.tile(idx_flat.reshape(total // 16, 16).T, (8, 1)).copy())
        dv_all.append(dv_flat.reshape(total // 128, 128).T.copy())
    return env, slot_off, total, idx_all, dv_all


def _raw_dma_gather(gp, out_ap, in_ap, idxs_ap, num_idxs, elem_size, elem_step):
    """dma_gather without the elem_size_bytes%256 assert (transpose-path-only
    restriction). Rows are elem_step-strided; each descriptor moves elem_size
    elements. Verified bit-exact on hardware for fp16 elem 64 / step 128."""
    import concourse.mybir as mybir
    import concourse.ap_utils as ap_utils
    assert in_ap.dtype == out_ap.dtype
    assert idxs_ap.dtype == mybir.dt.int16
    assert ap_utils.ap_is_contiguous(in_ap.ap[1:])
    assert ap_utils.ap_is_contiguous(out_ap.ap[1:])
    assert ap_utils.ap_is_contiguous(idxs_ap.ap[1:])
    assert in_ap.ap[-1][1] == out_ap.ap[-1][1] == elem_size
    assert in_ap.ap[0][0] == elem_step
    assert out_ap.ap[0][1] * out_ap.ap[1][1] == _round_up(num_idxs, 128)
    stride_bytes = elem_step * mybir.dt.size(in_ap.dtype)
    assert stride_bytes % 256 == 0 and stride_bytes // 256 < 256
    _in_ap = gp.lower_ap_dma(in_ap, for_custom_bir_dma=True)
    return gp.add_instruction(
        mybir.InstDMAGatherAnt(
            name=gp.bass.get_next_instruction_name(),
            ins=[*_in_ap, gp.lower_ap(idxs_ap),
                 gp.lower_val_access(gp.to_reg(num_idxs))],
            outs=[gp.lower_ap(out_ap)],
            transpose=False,
            num_idxs=num_idxs,
            elem_size=elem_size,
            stride_bytes_256=stride_bytes // 256,
            gen_mode=0,
            single_packet=False,
            queue_num=0,
        )
    )


def _build_program(env, slot_off, total):
    import concourse.bass as bass
    import concourse.bacc as bacc
    import concourse.mybir as mybir
    import concourse.tile as tile

    f32 = mybir.dt.float32
    fp16 = mybir.dt.float16

    sup_slots = []
    for s in range(N_SUPER):
        n = sum(int(env[s * G + tl, c])
                for c in range(N_CHUNKS) for tl in range(G_OF[s]))
        sup_slots.append(n)
    sup_off = np.concatenate([[0], np.cumsum(sup_slots)])[:-1].astype(np.int64)
    max_sup_slots = _round_up(max(sup_slots), 2048)
    max_seg_blocks = max(
        sum(int(env[s * G + tl, c]) for tl in range(G_OF[s])) // 128
        for s in range(N_SUPER) for c in range(N_CHUNKS))

    nc = bacc.Bacc("TRN2", target_bir_lowering=False, debug=False,
                   num_devices=N_CORES)
    w_d = nc.dram_tensor("w", [N_NODES, 128], fp16, kind="ExternalInput").ap()
    ci_d = nc.dram_tensor("ci", [128, N_TILES], f32, kind="ExternalInput").ap()
    io_d = nc.dram_tensor("io", [128, 128], fp16, kind="ExternalInput").ap()
    idx_d = nc.dram_tensor("idx", [128, total // 16], mybir.dt.int16,
                           kind="ExternalInput").ap()
    dv_d = nc.dram_tensor("dv", [128, total // 128], f32,
                          kind="ExternalInput").ap()
    out_d = nc.dram_tensor("out", [128, N_TILES * OUT_DIM], f32,
                           kind="ExternalOutput").ap()

    with tile.TileContext(nc) as tc:
        with (
            tc.tile_pool(name="const", bufs=1) as constp,
            tc.tile_pool(name="idxp", bufs=4) as idxp,
            tc.tile_pool(name="dvp", bufs=4) as dvp,
            tc.tile_pool(name="msg", bufs=12) as msgp,
            tc.tile_pool(name="oh", bufs=3) as ohp,
            tc.tile_pool(name="ps", bufs=8, space="PSUM") as psp,
            tc.tile_pool(name="ot", bufs=3) as otp,
        ):
            ci_t = constp.tile([128, N_TILES], f32)
            io_t = constp.tile([128, 128], fp16)
            nc.sync.dma_start(ci_t[:], ci_d[:])
            nc.sync.dma_start(io_t[:], io_d[:])

            for s in range(N_SUPER):
                gs = G_OF[s]
                base = int(sup_off[s])
                nsl = sup_slots[s]
                # stage this supertile's metadata (rotating pools provide
                # back-pressure so uploads interleave with earlier gathers)
                idx_t = idxp.tile([128, max_sup_slots // 16], mybir.dt.int16,
                                  tag="idx", name=f"idx_{s}")
                dv_t = dvp.tile([128, max_sup_slots // 128], f32,
                                tag="dv", name=f"dv_{s}")
                nc.sync.dma_start(idx_t[:, :nsl // 16],
                                  idx_d[:, base // 16:(base + nsl) // 16])
                nc.sync.dma_start(dv_t[:, :nsl // 128],
                                  dv_d[:, base // 128:(base + nsl) // 128])

                msgs = []
                rel = 0                              # slot offset within super
                seg_rel = []
                for c in range(N_CHUNKS):
                    n_sc = sum(int(env[s * G + tl, c]) for tl in range(gs))
                    msg = msgp.tile([128, max_seg_blocks, OUT_DIM], fp16,
                                    tag="msg", name=f"msg_{s}_{c}")
                    _raw_dma_gather(
                        nc.gpsimd, msg[:, :n_sc // 128, :],
                        w_d[c * CHUNK:(c + 1) * CHUNK, 0:OUT_DIM],
                        idx_t[:, rel // 16:(rel + n_sc) // 16],
                        n_sc, OUT_DIM, 128)
                    msgs.append(msg)
                    seg_rel.append(rel)
                    rel += n_sc

                n_blk_sup = nsl // 128
                oh_sup = ohp.tile([128, n_blk_sup, 128], fp16, tag="oh",
                                  name=f"oh_{s}", padded_shape=None)
                pss = [psp.tile([128, OUT_DIM], f32, tag="ps",
                                name=f"ps_{s}_{tl}") for tl in range(gs)]
                blk_total = [sum(int(env[s * G + tl, c])
                                 for c in range(N_CHUNKS)) // 128
                             for tl in range(gs)]
                blk_seen = [0] * gs
                for c in range(N_CHUNKS):
                    col = 0
                    for tl in range(gs):
                        n_blk_t = int(env[s * G + tl, c]) // 128
                        for b in range(n_blk_t):
                            gcol = (seg_rel[c] + 128 * (col + b)) // 128
                            nc.vector.tensor_scalar(
                                oh_sup[:, gcol, :], io_t[:],
                                dv_t[:, gcol:gcol + 1],
                                1.0, mybir.AluOpType.is_equal,
                                mybir.AluOpType.mult)
                            nc.tensor.matmul(
                                pss[tl][:], oh_sup[:, gcol, :],
                                msgs[c][:, col + b, :],
                                start=(blk_seen[tl] == 0),
                                stop=(blk_seen[tl] == blk_total[tl] - 1))
                            blk_seen[tl] += 1
                        col += n_blk_t

                ot = otp.tile([128, gs * OUT_DIM], f32, tag="ot",
                              name=f"ot_{s}")
                for tl in range(gs):
                    t = s * G + tl
                    nc.scalar.activation(
                        ot[:, tl * OUT_DIM:(tl + 1) * OUT_DIM], pss[tl][:],
                        mybir.ActivationFunctionType.Copy,
                        scale=ci_t[:, t:t + 1])
                nc.sync.dma_start(
                    out_d[:, s * G * OUT_DIM:(s * G + gs) * OUT_DIM], ot[:])

    nc.compile()
    return nc


def prepare(node_ids, src_idx, dst_idx, cj, ci, weight):
    """Host prep + program build. Returns (nc, in_maps, postprocess)."""
    import time
    _t0 = time.time()

    node_ids = np.asarray(node_ids)
    src = np.asarray(src_idx).astype(np.int64)
    dst = np.asarray(dst_idx).astype(np.int64)
    cj = np.asarray(cj, dtype=np.float32).reshape(-1)
    ci = np.asarray(ci, dtype=np.float32).reshape(-1)
    weight = np.asarray(weight, dtype=np.float32)

    # feat rows are weight[node_ids]; with the arange fill this is identity
    if not np.array_equal(node_ids, np.arange(N_NODES, dtype=node_ids.dtype)):
        weight = weight[node_ids]

    # prescale by cj and lay out as an fp16 table with 256B-strided rows
    w_tab = np.zeros((N_NODES, 128), np.float16)
    w_tab[:, :OUT_DIM] = (weight * cj[:, None]).astype(np.float16)

    iota = np.tile(np.arange(128, dtype=np.float16), (128, 1))

    env, slot_off, total, idx_all, dv_all = _host_prep(src, dst)
    print(f"[kernel] host prep: {time.time()-_t0:.1f}s (total slots {total})",
          flush=True)
    _t1 = time.time()
    nc = _build_program(env, slot_off, total)
    print(f"[kernel] build+schedule+compile-to-bir: {time.time()-_t1:.1f}s",
          flush=True)

    in_maps = []
    for k in range(N_CORES):
        ci_k = np.zeros(N_TILES * 128, np.float32)
        ci_k[:DST_PER_CORE] = ci[k * DST_PER_CORE:(k + 1) * DST_PER_CORE]
        ci_w = ci_k.reshape(N_TILES, 128).T.copy()
        in_maps.append({
            "w": w_tab, "ci": ci_w, "io": iota,
            "idx": idx_all[k], "dv": dv_all[k],
        })

    def post(results):
        outs = []
        for k in range(N_CORES):
            o = np.asarray(results[k]["out"])        # [128, 98*64]
            o = o.reshape(128, N_TILES, OUT_DIM).transpose(1, 0, 2)
            outs.append(o.reshape(-1, OUT_DIM)[:DST_PER_CORE])
        return np.concatenate(outs, axis=0)

    return nc, in_maps, post


def kernel(node_ids, src_idx, dst_idx, cj, ci, weight):
    import time
    from concourse.bass_utils import run_bass_kernel_spmd
    nc, in_maps, post = prepare(node_ids, src_idx, dst_idx, cj, ci, weight)
    _t2 = time.time()
    res = run_bass_kernel_spmd(nc, in_maps, core_ids=list(range(N_CORES)))
    print(f"[kernel] neff compile+exec: {time.time()-_t2:.1f}s", flush=True)
    return post(res.results)


# revision 10
# speedup vs baseline: 1.6109x; 1.0394x over previous
"""GCMC graph-conv kernel for Trainium2, 8-core SPMD.

out = ci * segment_sum((weight[node_ids] * cj)[src_idx], dst_idx)

Strategy (edge sharding by dst range, fp16 message path):
  - host prescales W' = weight[node_ids] * cj, stores it as an fp16 table with
    256B-strided rows ([100000, 128] fp16, data in cols 0:64) so each gather
    descriptor moves only 128B (half the DMA time of a 256B fp32 row)
  - core k owns dst rows [k*12500, (k+1)*12500); its edges are partitioned by
    (supertile of G=4 dst tiles, src chunk of 25000, dst tile) with each
    (supertile, chunk, tile) sub-segment padded to 128 slots using a shared
    static envelope (max over cores) so the program is SPMD-identical
  - one SWDGE dma_gather per (supertile, chunk) — 100 gathers instead of 392 —
    emitted raw (the bass wrapper's elem%256 assert is a transpose-path
    restriction; elem_step=128/elem_size=64 fp16 is valid and verified on hw)
  - segment-sum via one-hot matmul on DVE+PE: oh[slot, d] = (iota[d] ==
    dv[slot]); psum[dst, 64] += oh.T @ msg in fp16; psum groups are
    double-buffered (2 supertiles x 4 banks) so one supertile's tail overlaps
    the next's head
  - idx/dv metadata staged through rotating pools inside the supertile loop so
    uploads interleave with gathers instead of front-loading the DMA queue
  - flush: ACT copies psum*ci into a per-supertile staging tile, one HWDGE
    DMA per supertile writes [128, G*64] fp32 to a partition-major output
    buffer that the host untransposes
"""
import sys, os
sys.path.insert(0, '/opt/trn_rl_repo')

import numpy as np

N_NODES = 100000
OUT_DIM = 64
N_CORES = 8
DST_PER_CORE = N_NODES // N_CORES          # 12500
N_TILES = (DST_PER_CORE + 127) // 128      # 98
G = 4                                      # dst tiles per supertile
N_SUPER = (N_TILES + G - 1) // G           # 25 (last has 2 tiles)
G_OF = [min(G, N_TILES - s * G) for s in range(N_SUPER)]
# int16 gather indices allow chunks up to 32767 rows; uneven chunks put the
# per-(tile, chunk) 128-rounding waste into 3 big cells + 1 small one
CHUNK_BASE = [0, 32767, 65534, 98301]
CHUNK_SIZE = [32767, 32767, 32767, 1699]
N_CHUNKS = 4
PAD_SENTINEL = 999.0


def _round_up(x, m):
    return (x + m - 1) // m * m


def _host_prep(src, dst):
    """Partition edges by dst core range; compute the shared static envelope
    env[t, c] (max per-core (tile, chunk) count, rounded to 128) and per-core
    slot-packed idx / dv arrays laid out in envelope slots ordered by
    (supertile, chunk, tile)."""
    per_core = []
    counts = np.zeros((N_CORES, N_TILES, N_CHUNKS), np.int64)
    for k in range(N_CORES):
        m = (dst // DST_PER_CORE) == k
        s_e = src[m]
        dl = dst[m] - k * DST_PER_CORE
        t = dl >> 7                         # dst tile 0..97
        c = np.minimum(s_e // 32767, 3)     # src chunk 0..3
        counts[k] = np.bincount(t * N_CHUNKS + c,
                                minlength=N_TILES * N_CHUNKS).reshape(
                                    N_TILES, N_CHUNKS)
        per_core.append((s_e, dl, t, c))

    env = _round_up(counts.max(axis=0), 128)       # [T, C]
    # slot order: (supertile, chunk, tile)
    grp_key = []
    for s in range(N_SUPER):
        for c in range(N_CHUNKS):
            for tl in range(G_OF[s]):
                grp_key.append((s * G + tl, c))
    env_seq = np.array([env[t, c] for (t, c) in grp_key], np.int64)
    slot_off_seq = np.concatenate([[0], np.cumsum(env_seq)])[:-1]
    total = int(env_seq.sum())
    slot_off = np.zeros((N_TILES, N_CHUNKS), np.int64)
    for g, (t, c) in enumerate(grp_key):
        slot_off[t, c] = slot_off_seq[g]

    idx_all, dv_all = [], []
    for k in range(N_CORES):
        s_e, dl, t, c = per_core[k]
        gid = t * N_CHUNKS + c
        order = np.argsort(gid, kind='stable')
        s_e, dl, t, c, gid = (s_e[order], dl[order], t[order], c[order],
                              gid[order])
        gcounts = np.bincount(gid, minlength=N_TILES * N_CHUNKS)
        within = np.arange(len(s_e)) - np.repeat(
            np.concatenate([[0], np.cumsum(gcounts)])[:-1], gcounts)
        slot = slot_off[t, c] + within
        idx_flat = np.zeros(total, np.int16)       # pad slots gather row 0
        idx_flat[slot] = (s_e - np.asarray(CHUNK_BASE)[c]).astype(np.int16)
        dv_flat = np.full(total, PAD_SENTINEL, np.float32)
        dv_flat[slot] = (dl & 127).astype(np.float32)
        # idx wrapped into 16 partitions, replicated x8 (one copy per Q7 core)
        idx_all.append(np.tile(idx_flat.reshape(total // 16, 16).T, (8, 1)).copy())
        dv_all.append(dv_flat.reshape(total // 128, 128).T.copy())
    return env, slot_off, total, idx_all, dv_all


def _raw_dma_gather(gp, out_ap, in_ap, idxs_ap, num_idxs, elem_size, elem_step):
    """dma_gather without the elem_size_bytes%256 assert (transpose-path-only
    restriction). Rows are elem_step-strided; each descriptor moves elem_size
    elements. Verified bit-exact on hardware for fp16 elem 64 / step 128."""
    import concourse.mybir as mybir
    import concourse.ap_utils as ap_utils
    assert in_ap.dtype == out_ap.dtype
    assert idxs_ap.dtype == mybir.dt.int16
    assert ap_utils.ap_is_contiguous(in_ap.ap[1:])
    assert ap_utils.ap_is_contiguous(out_ap.ap[1:])
    assert ap_utils.ap_is_contiguous(idxs_ap.ap[1:])
    assert in_ap.ap[-1][1] == out_ap.ap[-1][1] == elem_size
    assert in_ap.ap[0][0] == elem_step
    assert out_ap.ap[0][1] * out_ap.ap[1][1] == _round_up(num_idxs, 128)
    stride_bytes = elem_step * mybir.dt.size(in_ap.dtype)
    assert stride_bytes % 256 == 0 and stride_bytes // 256 < 256
    _in_ap = gp.lower_ap_dma(in_ap, for_custom_bir_dma=True)
    return gp.add_instruction(
        mybir.InstDMAGatherAnt(
            name=gp.bass.get_next_instruction_name(),
            ins=[*_in_ap, gp.lower_ap(idxs_ap),
                 gp.lower_val_access(gp.to_reg(num_idxs))],
            outs=[gp.lower_ap(out_ap)],
            transpose=False,
            num_idxs=num_idxs,
            elem_size=elem_size,
            stride_bytes_256=stride_bytes // 256,
            gen_mode=0,
            single_packet=False,
            queue_num=0,
        )
    )


def _build_program(env, slot_off, total):
    import concourse.bass as bass
    import concourse.bacc as bacc
    import concourse.mybir as mybir
    import concourse.tile as tile

    f32 = mybir.dt.float32
    fp16 = mybir.dt.float16

    sup_slots = []
    for s in range(N_SUPER):
        n = sum(int(env[s * G + tl, c])
                for c in range(N_CHUNKS) for tl in range(G_OF[s]))
        sup_slots.append(n)
    sup_off = np.concatenate([[0], np.cumsum(sup_slots)])[:-1].astype(np.int64)
    max_sup_slots = _round_up(max(sup_slots), 2048)
    max_seg_blocks = max(
        sum(int(env[s * G + tl, c]) for tl in range(G_OF[s])) // 128
        for s in range(N_SUPER) for c in range(N_CHUNKS))

    nc = bacc.Bacc("TRN2", target_bir_lowering=False, debug=False,
                   num_devices=N_CORES)
    w_d = nc.dram_tensor("w", [N_NODES, 128], fp16, kind="ExternalInput").ap()
    ci_d = nc.dram_tensor("ci", [128, N_TILES], f32, kind="ExternalInput").ap()
    io_d = nc.dram_tensor("io", [128, 128], fp16, kind="ExternalInput").ap()
    idx_d = nc.dram_tensor("idx", [128, total // 16], mybir.dt.int16,
                           kind="ExternalInput").ap()
    dv_d = nc.dram_tensor("dv", [128, total // 128], f32,
                          kind="ExternalInput").ap()
    out_d = nc.dram_tensor("out", [128, N_TILES * OUT_DIM], f32,
                           kind="ExternalOutput").ap()

    with tile.TileContext(nc) as tc:
        with (
            tc.tile_pool(name="const", bufs=1) as constp,
            tc.tile_pool(name="idxp", bufs=4) as idxp,
            tc.tile_pool(name="dvp", bufs=4) as dvp,
            tc.tile_pool(name="msg", bufs=12) as msgp,
            tc.tile_pool(name="oh", bufs=3) as ohp,
            tc.tile_pool(name="ps", bufs=8, space="PSUM") as psp,
            tc.tile_pool(name="ot", bufs=3) as otp,
        ):
            ci_t = constp.tile([128, N_TILES], f32)
            io_t = constp.tile([128, 128], fp16)
            nc.sync.dma_start(ci_t[:], ci_d[:])
            nc.sync.dma_start(io_t[:], io_d[:])

            for s in range(N_SUPER):
                gs = G_OF[s]
                base = int(sup_off[s])
                nsl = sup_slots[s]
                # stage this supertile's metadata (rotating pools provide
                # back-pressure so uploads interleave with earlier gathers)
                idx_t = idxp.tile([128, max_sup_slots // 16], mybir.dt.int16,
                                  tag="idx", name=f"idx_{s}")
                dv_t = dvp.tile([128, max_sup_slots // 128], f32,
                                tag="dv", name=f"dv_{s}")
                nc.sync.dma_start(idx_t[:, :nsl // 16],
                                  idx_d[:, base // 16:(base + nsl) // 16])
                nc.sync.dma_start(dv_t[:, :nsl // 128],
                                  dv_d[:, base // 128:(base + nsl) // 128])

                msgs = []
                rel = 0                              # slot offset within super
                seg_rel = []
                for c in range(N_CHUNKS):
                    n_sc = sum(int(env[s * G + tl, c]) for tl in range(gs))
                    msg = msgp.tile([128, max_seg_blocks, OUT_DIM], fp16,
                                    tag="msg", name=f"msg_{s}_{c}")
                    _raw_dma_gather(
                        nc.gpsimd, msg[:, :n_sc // 128, :],
                        w_d[CHUNK_BASE[c]:CHUNK_BASE[c] + CHUNK_SIZE[c],
                            0:OUT_DIM],
                        idx_t[:, rel // 16:(rel + n_sc) // 16],
                        n_sc, OUT_DIM, 128)
                    msgs.append(msg)
                    seg_rel.append(rel)
                    rel += n_sc

                n_blk_sup = nsl // 128
                oh_sup = ohp.tile([128, n_blk_sup, 128], fp16, tag="oh",
                                  name=f"oh_{s}", padded_shape=None)
                pss = [psp.tile([128, OUT_DIM], f32, tag="ps",
                                name=f"ps_{s}_{tl}") for tl in range(gs)]
                blk_total = [sum(int(env[s * G + tl, c])
                                 for c in range(N_CHUNKS)) // 128
                             for tl in range(gs)]
                blk_seen = [0] * gs
                for c in range(N_CHUNKS):
                    col = 0
                    for tl in range(gs):
                        n_blk_t = int(env[s * G + tl, c]) // 128
                        for b in range(n_blk_t):
                            gcol = (seg_rel[c] + 128 * (col + b)) // 128
                            nc.vector.tensor_scalar(
                                oh_sup[:, gcol, :], io_t[:],
                                dv_t[:, gcol:gcol + 1],
                                1.0, mybir.AluOpType.is_equal,
                                mybir.AluOpType.mult)
                            nc.tensor.matmul(
                                pss[tl][:], oh_sup[:, gcol, :],
                                msgs[c][:, col + b, :],
                                start=(blk_seen[tl] == 0),
                                stop=(blk_seen[tl] == blk_total[tl] - 1))
                            blk_seen[tl] += 1
                        col += n_blk_t

                ot = otp.tile([128, gs * OUT_DIM], f32, tag="ot",
                              name=f"ot_{s}")
                for tl in range(gs):
                    t = s * G + tl
                    nc.scalar.activation(
                        ot[:, tl * OUT_DIM:(tl + 1) * OUT_DIM], pss[tl][:],
                        mybir.ActivationFunctionType.Copy,
                        scale=ci_t[:, t:t + 1])
                nc.sync.dma_start(
                    out_d[:, s * G * OUT_DIM:(s * G + gs) * OUT_DIM], ot[:])

    nc.compile()
    return nc


def prepare(node_ids, src_idx, dst_idx, cj, ci, weight):
    """Host prep + program build. Returns (nc, in_maps, postprocess)."""
    import time
    _t0 = time.time()

    node_ids = np.asarray(node_ids)
    src = np.asarray(src_idx).astype(np.int64)
    dst = np.asarray(dst_idx).astype(np.int64)
    cj = np.asarray(cj, dtype=np.float32).reshape(-1)
    ci = np.asarray(ci, dtype=np.float32).reshape(-1)
    weight = np.asarray(weight, dtype=np.float32)

    # feat rows are weight[node_ids]; with the arange fill this is identity
    if not np.array_equal(node_ids, np.arange(N_NODES, dtype=node_ids.dtype)):
        weight = weight[node_ids]

    # prescale by cj and lay out as an fp16 table with 256B-strided rows
    w_tab = np.zeros((N_NODES, 128), np.float16)
    w_tab[:, :OUT_DIM] = (weight * cj[:, None]).astype(np.float16)

    iota = np.tile(np.arange(128, dtype=np.float16), (128, 1))

    env, slot_off, total, idx_all, dv_all = _host_prep(src, dst)
    print(f"[kernel] host prep: {time.time()-_t0:.1f}s (total slots {total})",
          flush=True)
    _t1 = time.time()
    nc = _build_program(env, slot_off, total)
    print(f"[kernel] build+schedule+compile-to-bir: {time.time()-_t1:.1f}s",
          flush=True)

    in_maps = []
    for k in range(N_CORES):
        ci_k = np.zeros(N_TILES * 128, np.float32)
        ci_k[:DST_PER_CORE] = ci[k * DST_PER_CORE:(k + 1) * DST_PER_CORE]
        ci_w = ci_k.reshape(N_TILES, 128).T.copy()
        in_maps.append({
            "w": w_tab, "ci": ci_w, "io": iota,
            "idx": idx_all[k], "dv": dv_all[k],
        })

    def post(results):
        outs = []
        for k in range(N_CORES):
            o = np.asarray(results[k]["out"])        # [128, 98*64]
            o = o.reshape(128, N_TILES, OUT_DIM).transpose(1, 0, 2)
            outs.append(o.reshape(-1, OUT_DIM)[:DST_PER_CORE])
        return np.concatenate(outs, axis=0)

    return nc, in_maps, post


def kernel(node_ids, src_idx, dst_idx, cj, ci, weight):
    import time
    from concourse.bass_utils import run_bass_kernel_spmd
    nc, in_maps, post = prepare(node_ids, src_idx, dst_idx, cj, ci, weight)
    _t2 = time.time()
    res = run_bass_kernel_spmd(nc, in_maps, core_ids=list(range(N_CORES)))
    print(f"[kernel] neff compile+exec: {time.time()-_t2:.1f}s", flush=True)
    return post(res.results)


# revision 11
# speedup vs baseline: 1.6702x; 1.0368x over previous
"""GCMC graph-conv kernel for Trainium2, 8-core SPMD.

out = ci * segment_sum((weight[node_ids] * cj)[src_idx], dst_idx)

Strategy (edge sharding by dst range, fp16 message path):
  - host prescales W' = weight[node_ids] * cj, stores it as an fp16 table with
    256B-strided rows ([100000, 128] fp16, data in cols 0:64) so each gather
    descriptor moves only 128B (half the DMA time of a 256B fp32 row)
  - core k owns dst rows [k*12500, (k+1)*12500); its edges are partitioned by
    (supertile of G=4 dst tiles, src chunk of 25000, dst tile) with each
    (supertile, chunk, tile) sub-segment padded to 128 slots using a shared
    static envelope (max over cores) so the program is SPMD-identical
  - one SWDGE dma_gather per (supertile, chunk) — 100 gathers instead of 392 —
    emitted raw (the bass wrapper's elem%256 assert is a transpose-path
    restriction; elem_step=128/elem_size=64 fp16 is valid and verified on hw)
  - segment-sum via one-hot matmul on DVE+PE: oh[slot, d] = (iota[d] ==
    dv[slot]); psum[dst, 64] += oh.T @ msg in fp16; psum groups are
    double-buffered (2 supertiles x 4 banks) so one supertile's tail overlaps
    the next's head
  - idx/dv metadata staged through rotating pools inside the supertile loop so
    uploads interleave with gathers instead of front-loading the DMA queue
  - flush: ACT copies psum*ci into a per-supertile staging tile, one HWDGE
    DMA per supertile writes [128, G*64] fp32 to a partition-major output
    buffer that the host untransposes
"""
import sys, os
sys.path.insert(0, '/opt/trn_rl_repo')

import numpy as np

N_NODES = 100000
OUT_DIM = 64
N_CORES = 8
DST_PER_CORE = N_NODES // N_CORES          # 12500
N_TILES = (DST_PER_CORE + 127) // 128      # 98
G = 4                                      # dst tiles per supertile
N_SUPER = (N_TILES + G - 1) // G           # 25 (last has 2 tiles)
G_OF = [min(G, N_TILES - s * G) for s in range(N_SUPER)]
# int16 gather indices allow chunks up to 32767 rows; uneven chunks put the
# per-(tile, chunk) 128-rounding waste into 3 big cells + 1 small one
CHUNK_BASE = [0, 32767, 65534, 98301]
CHUNK_SIZE = [32767, 32767, 32767, 1699]
N_CHUNKS = 4
PAD_SENTINEL = 999.0


def _round_up(x, m):
    return (x + m - 1) // m * m


def _host_prep(src, dst):
    """Partition edges by dst core range; compute the shared static envelope
    env[t, c] (max per-core (tile, chunk) count, rounded to 128) and per-core
    slot-packed idx / dv arrays laid out in envelope slots ordered by
    (supertile, chunk, tile)."""
    per_core = []
    counts = np.zeros((N_CORES, N_TILES, N_CHUNKS), np.int64)
    for k in range(N_CORES):
        m = (dst // DST_PER_CORE) == k
        s_e = src[m]
        dl = dst[m] - k * DST_PER_CORE
        t = dl >> 7                         # dst tile 0..97
        c = np.minimum(s_e // 32767, 3)     # src chunk 0..3
        counts[k] = np.bincount(t * N_CHUNKS + c,
                                minlength=N_TILES * N_CHUNKS).reshape(
                                    N_TILES, N_CHUNKS)
        per_core.append((s_e, dl, t, c))

    env = _round_up(counts.max(axis=0), 128)       # [T, C]
    # slot order: (supertile, chunk, tile)
    grp_key = []
    for s in range(N_SUPER):
        for c in range(N_CHUNKS):
            for tl in range(G_OF[s]):
                grp_key.append((s * G + tl, c))
    env_seq = np.array([env[t, c] for (t, c) in grp_key], np.int64)
    slot_off_seq = np.concatenate([[0], np.cumsum(env_seq)])[:-1]
    total = int(env_seq.sum())
    slot_off = np.zeros((N_TILES, N_CHUNKS), np.int64)
    for g, (t, c) in enumerate(grp_key):
        slot_off[t, c] = slot_off_seq[g]

    idx_all, dv_all = [], []
    for k in range(N_CORES):
        s_e, dl, t, c = per_core[k]
        gid = t * N_CHUNKS + c
        order = np.argsort(gid, kind='stable')
        s_e, dl, t, c, gid = (s_e[order], dl[order], t[order], c[order],
                              gid[order])
        gcounts = np.bincount(gid, minlength=N_TILES * N_CHUNKS)
        within = np.arange(len(s_e)) - np.repeat(
            np.concatenate([[0], np.cumsum(gcounts)])[:-1], gcounts)
        slot = slot_off[t, c] + within
        idx_flat = np.zeros(total, np.int16)       # pad slots gather row 0
        idx_flat[slot] = (s_e - np.asarray(CHUNK_BASE)[c]).astype(np.int16)
        dv_flat = np.full(total, PAD_SENTINEL, np.float32)
        dv_flat[slot] = (dl & 127).astype(np.float32)
        # idx wrapped into 16 partitions; the gather ucode only reads
        # partitions 16:32 (probed on hw, deterministic across cores), so
        # upload just bands 0+1 (band 0 also covers the interp's read path)
        idx_all.append(np.tile(idx_flat.reshape(total // 16, 16).T, (2, 1)).copy())
        dv_all.append(dv_flat.reshape(total // 128, 128).T.copy())
    return env, slot_off, total, idx_all, dv_all


def _raw_dma_gather(gp, out_ap, in_ap, idxs_ap, num_idxs, elem_size, elem_step):
    """dma_gather without the elem_size_bytes%256 assert (transpose-path-only
    restriction). Rows are elem_step-strided; each descriptor moves elem_size
    elements. Verified bit-exact on hardware for fp16 elem 64 / step 128."""
    import concourse.mybir as mybir
    import concourse.ap_utils as ap_utils
    assert in_ap.dtype == out_ap.dtype
    assert idxs_ap.dtype == mybir.dt.int16
    assert ap_utils.ap_is_contiguous(in_ap.ap[1:])
    assert ap_utils.ap_is_contiguous(out_ap.ap[1:])
    assert ap_utils.ap_is_contiguous(idxs_ap.ap[1:])
    assert in_ap.ap[-1][1] == out_ap.ap[-1][1] == elem_size
    assert in_ap.ap[0][0] == elem_step
    assert out_ap.ap[0][1] * out_ap.ap[1][1] == _round_up(num_idxs, 128)
    stride_bytes = elem_step * mybir.dt.size(in_ap.dtype)
    assert stride_bytes % 256 == 0 and stride_bytes // 256 < 256
    _in_ap = gp.lower_ap_dma(in_ap, for_custom_bir_dma=True)
    return gp.add_instruction(
        mybir.InstDMAGatherAnt(
            name=gp.bass.get_next_instruction_name(),
            ins=[*_in_ap, gp.lower_ap(idxs_ap),
                 gp.lower_val_access(gp.to_reg(num_idxs))],
            outs=[gp.lower_ap(out_ap)],
            transpose=False,
            num_idxs=num_idxs,
            elem_size=elem_size,
            stride_bytes_256=stride_bytes // 256,
            gen_mode=0,
            single_packet=False,
            queue_num=0,
        )
    )


def _build_program(env, slot_off, total):
    import concourse.bass as bass
    import concourse.bacc as bacc
    import concourse.mybir as mybir
    import concourse.tile as tile

    f32 = mybir.dt.float32
    fp16 = mybir.dt.float16

    sup_slots = []
    for s in range(N_SUPER):
        n = sum(int(env[s * G + tl, c])
                for c in range(N_CHUNKS) for tl in range(G_OF[s]))
        sup_slots.append(n)
    sup_off = np.concatenate([[0], np.cumsum(sup_slots)])[:-1].astype(np.int64)
    max_sup_slots = _round_up(max(sup_slots), 2048)
    max_seg_blocks = max(
        sum(int(env[s * G + tl, c]) for tl in range(G_OF[s])) // 128
        for s in range(N_SUPER) for c in range(N_CHUNKS))

    nc = bacc.Bacc("TRN2", target_bir_lowering=False, debug=False,
                   num_devices=N_CORES)
    w_d = nc.dram_tensor("w", [N_NODES, 128], fp16, kind="ExternalInput").ap()
    ci_d = nc.dram_tensor("ci", [128, N_TILES], f32, kind="ExternalInput").ap()
    io_d = nc.dram_tensor("io", [128, 128], fp16, kind="ExternalInput").ap()
    idx_d = nc.dram_tensor("idx", [32, total // 16], mybir.dt.int16,
                           kind="ExternalInput").ap()
    dv_d = nc.dram_tensor("dv", [128, total // 128], f32,
                          kind="ExternalInput").ap()
    out_d = nc.dram_tensor("out", [128, N_TILES * OUT_DIM], f32,
                           kind="ExternalOutput").ap()

    with tile.TileContext(nc) as tc:
        with (
            tc.tile_pool(name="const", bufs=1) as constp,
            tc.tile_pool(name="idxp", bufs=4) as idxp,
            tc.tile_pool(name="dvp", bufs=4) as dvp,
            tc.tile_pool(name="msg", bufs=12) as msgp,
            tc.tile_pool(name="oh", bufs=3) as ohp,
            tc.tile_pool(name="ps", bufs=8, space="PSUM") as psp,
            tc.tile_pool(name="ot", bufs=3) as otp,
        ):
            ci_t = constp.tile([128, N_TILES], f32)
            io_t = constp.tile([128, 128], fp16)
            nc.sync.dma_start(ci_t[:], ci_d[:])
            nc.sync.dma_start(io_t[:], io_d[:])

            for s in range(N_SUPER):
                gs = G_OF[s]
                base = int(sup_off[s])
                nsl = sup_slots[s]
                # stage this supertile's metadata (rotating pools provide
                # back-pressure so uploads interleave with earlier gathers)
                idx_t = idxp.tile([128, max_sup_slots // 16], mybir.dt.int16,
                                  tag="idx", name=f"idx_{s}")
                dv_t = dvp.tile([128, max_sup_slots // 128], f32,
                                tag="dv", name=f"dv_{s}")
                nc.sync.dma_start(idx_t[0:32, :nsl // 16],
                                  idx_d[:, base // 16:(base + nsl) // 16])
                nc.sync.dma_start(dv_t[:, :nsl // 128],
                                  dv_d[:, base // 128:(base + nsl) // 128])

                msgs = []
                rel = 0                              # slot offset within super
                seg_rel = []
                for c in range(N_CHUNKS):
                    n_sc = sum(int(env[s * G + tl, c]) for tl in range(gs))
                    msg = msgp.tile([128, max_seg_blocks, OUT_DIM], fp16,
                                    tag="msg", name=f"msg_{s}_{c}")
                    _raw_dma_gather(
                        nc.gpsimd, msg[:, :n_sc // 128, :],
                        w_d[CHUNK_BASE[c]:CHUNK_BASE[c] + CHUNK_SIZE[c],
                            0:OUT_DIM],
                        idx_t[:, rel // 16:(rel + n_sc) // 16],
                        n_sc, OUT_DIM, 128)
                    msgs.append(msg)
                    seg_rel.append(rel)
                    rel += n_sc

                n_blk_sup = nsl // 128
                oh_sup = ohp.tile([128, n_blk_sup, 128], fp16, tag="oh",
                                  name=f"oh_{s}", padded_shape=None)
                pss = [psp.tile([128, OUT_DIM], f32, tag="ps",
                                name=f"ps_{s}_{tl}") for tl in range(gs)]
                blk_total = [sum(int(env[s * G + tl, c])
                                 for c in range(N_CHUNKS)) // 128
                             for tl in range(gs)]
                blk_seen = [0] * gs
                for c in range(N_CHUNKS):
                    col = 0
                    for tl in range(gs):
                        n_blk_t = int(env[s * G + tl, c]) // 128
                        for b in range(n_blk_t):
                            gcol = (seg_rel[c] + 128 * (col + b)) // 128
                            nc.vector.tensor_scalar(
                                oh_sup[:, gcol, :], io_t[:],
                                dv_t[:, gcol:gcol + 1],
                                1.0, mybir.AluOpType.is_equal,
                                mybir.AluOpType.mult)
                            nc.tensor.matmul(
                                pss[tl][:], oh_sup[:, gcol, :],
                                msgs[c][:, col + b, :],
                                start=(blk_seen[tl] == 0),
                                stop=(blk_seen[tl] == blk_total[tl] - 1))
                            blk_seen[tl] += 1
                        col += n_blk_t

                ot = otp.tile([128, gs * OUT_DIM], f32, tag="ot",
                              name=f"ot_{s}")
                for tl in range(gs):
                    t = s * G + tl
                    nc.scalar.activation(
                        ot[:, tl * OUT_DIM:(tl + 1) * OUT_DIM], pss[tl][:],
                        mybir.ActivationFunctionType.Copy,
                        scale=ci_t[:, t:t + 1])
                nc.sync.dma_start(
                    out_d[:, s * G * OUT_DIM:(s * G + gs) * OUT_DIM], ot[:])

    nc.compile()
    return nc


def prepare(node_ids, src_idx, dst_idx, cj, ci, weight):
    """Host prep + program build. Returns (nc, in_maps, postprocess)."""
    import time
    _t0 = time.time()

    node_ids = np.asarray(node_ids)
    src = np.asarray(src_idx).astype(np.int64)
    dst = np.asarray(dst_idx).astype(np.int64)
    cj = np.asarray(cj, dtype=np.float32).reshape(-1)
    ci = np.asarray(ci, dtype=np.float32).reshape(-1)
    weight = np.asarray(weight, dtype=np.float32)

    # feat rows are weight[node_ids]; with the arange fill this is identity
    if not np.array_equal(node_ids, np.arange(N_NODES, dtype=node_ids.dtype)):
        weight = weight[node_ids]

    # prescale by cj and lay out as an fp16 table with 256B-strided rows
    w_tab = np.zeros((N_NODES, 128), np.float16)
    w_tab[:, :OUT_DIM] = (weight * cj[:, None]).astype(np.float16)

    iota = np.tile(np.arange(128, dtype=np.float16), (128, 1))

    env, slot_off, total, idx_all, dv_all = _host_prep(src, dst)
    print(f"[kernel] host prep: {time.time()-_t0:.1f}s (total slots {total})",
          flush=True)
    _t1 = time.time()
    nc = _build_program(env, slot_off, total)
    print(f"[kernel] build+schedule+compile-to-bir: {time.time()-_t1:.1f}s",
          flush=True)

    in_maps = []
    for k in range(N_CORES):
        ci_k = np.zeros(N_TILES * 128, np.float32)
        ci_k[:DST_PER_CORE] = ci[k * DST_PER_CORE:(k + 1) * DST_PER_CORE]
        ci_w = ci_k.reshape(N_TILES, 128).T.copy()
        in_maps.append({
            "w": w_tab, "ci": ci_w, "io": iota,
            "idx": idx_all[k], "dv": dv_all[k],
        })

    def post(results):
        outs = []
        for k in range(N_CORES):
            o = np.asarray(results[k]["out"])        # [128, 98*64]
            o = o.reshape(128, N_TILES, OUT_DIM).transpose(1, 0, 2)
            outs.append(o.reshape(-1, OUT_DIM)[:DST_PER_CORE])
        return np.concatenate(outs, axis=0)

    return nc, in_maps, post


def kernel(node_ids, src_idx, dst_idx, cj, ci, weight):
    import time
    from concourse.bass_utils import run_bass_kernel_spmd
    nc, in_maps, post = prepare(node_ids, src_idx, dst_idx, cj, ci, weight)
    _t2 = time.time()
    res = run_bass_kernel_spmd(nc, in_maps, core_ids=list(range(N_CORES)))
    print(f"[kernel] neff compile+exec: {time.time()-_t2:.1f}s", flush=True)
    return post(res.results)


# revision 13
# speedup vs baseline: 1.7260x; 1.0334x over previous
"""GCMC graph-conv kernel for Trainium2, 8-core SPMD.

out = ci * segment_sum((weight[node_ids] * cj)[src_idx], dst_idx)

Strategy (edge sharding by dst range, fp16 message path):
  - host prescales W' = weight[node_ids] * cj, stores it as an fp16 table with
    256B-strided rows ([100000, 128] fp16, data in cols 0:64) so each gather
    descriptor moves only 128B (half the DMA time of a 256B fp32 row)
  - core k owns dst rows [k*12500, (k+1)*12500); its edges are partitioned by
    (supertile of G=4 dst tiles, src chunk of 25000, dst tile) with each
    (supertile, chunk, tile) sub-segment padded to 128 slots using a shared
    static envelope (max over cores) so the program is SPMD-identical
  - one SWDGE dma_gather per (supertile, chunk) — 100 gathers instead of 392 —
    emitted raw (the bass wrapper's elem%256 assert is a transpose-path
    restriction; elem_step=128/elem_size=64 fp16 is valid and verified on hw)
  - segment-sum via one-hot matmul on DVE+PE: oh[slot, d] = (iota[d] ==
    dv[slot]); psum[dst, 64] += oh.T @ msg in fp16; psum groups are
    double-buffered (2 supertiles x 4 banks) so one supertile's tail overlaps
    the next's head
  - idx/dv metadata staged through rotating pools inside the supertile loop so
    uploads interleave with gathers instead of front-loading the DMA queue
  - flush: ACT copies psum*ci into a per-supertile staging tile, one HWDGE
    DMA per supertile writes [128, G*64] fp32 to a partition-major output
    buffer that the host untransposes
"""
import sys, os
sys.path.insert(0, '/opt/trn_rl_repo')

import numpy as np

N_NODES = 100000
OUT_DIM = 64
N_CORES = 8
DST_PER_CORE = N_NODES // N_CORES          # 12500
N_TILES = (DST_PER_CORE + 127) // 128      # 98
G = 4                                      # dst tiles per supertile
N_SUPER = (N_TILES + G - 1) // G           # 25 (last has 2 tiles)
G_OF = [min(G, N_TILES - s * G) for s in range(N_SUPER)]
# int16 gather indices allow chunks up to 32767 rows; uneven chunks put the
# per-(tile, chunk) 128-rounding waste into 3 big cells + 1 small one
CHUNK_BASE = [0, 32767, 65534, 98301]
CHUNK_SIZE = [32767, 32767, 32767, 1699]
N_CHUNKS = 4
PAD_SENTINEL = 999.0


def _round_up(x, m):
    return (x + m - 1) // m * m


def _host_prep(src, dst):
    """Partition edges by dst core range; compute the shared static envelope
    env[t, c] (max per-core (tile, chunk) count, rounded to 128) and per-core
    slot-packed idx / dv arrays laid out in envelope slots ordered by
    (supertile, chunk, tile)."""
    per_core = []
    counts = np.zeros((N_CORES, N_TILES, N_CHUNKS), np.int64)
    for k in range(N_CORES):
        m = (dst // DST_PER_CORE) == k
        s_e = src[m]
        dl = dst[m] - k * DST_PER_CORE
        t = dl >> 7                         # dst tile 0..97
        c = np.minimum(s_e // 32767, 3)     # src chunk 0..3
        counts[k] = np.bincount(t * N_CHUNKS + c,
                                minlength=N_TILES * N_CHUNKS).reshape(
                                    N_TILES, N_CHUNKS)
        per_core.append((s_e, dl, t, c))

    env = _round_up(counts.max(axis=0), 128)       # [T, C]
    # slot order: (supertile, chunk, tile)
    grp_key = []
    for s in range(N_SUPER):
        for c in range(N_CHUNKS):
            for tl in range(G_OF[s]):
                grp_key.append((s * G + tl, c))
    env_seq = np.array([env[t, c] for (t, c) in grp_key], np.int64)
    slot_off_seq = np.concatenate([[0], np.cumsum(env_seq)])[:-1]
    total = int(env_seq.sum())
    slot_off = np.zeros((N_TILES, N_CHUNKS), np.int64)
    for g, (t, c) in enumerate(grp_key):
        slot_off[t, c] = slot_off_seq[g]

    idx_all, dv_all = [], []
    for k in range(N_CORES):
        s_e, dl, t, c = per_core[k]
        gid = t * N_CHUNKS + c
        order = np.argsort(gid, kind='stable')
        s_e, dl, t, c, gid = (s_e[order], dl[order], t[order], c[order],
                              gid[order])
        gcounts = np.bincount(gid, minlength=N_TILES * N_CHUNKS)
        within = np.arange(len(s_e)) - np.repeat(
            np.concatenate([[0], np.cumsum(gcounts)])[:-1], gcounts)
        slot = slot_off[t, c] + within
        idx_flat = np.zeros(total, np.int16)       # pad slots gather row 0
        idx_flat[slot] = (s_e - np.asarray(CHUNK_BASE)[c]).astype(np.int16)
        dv_flat = np.full(total, PAD_SENTINEL, np.float32)
        dv_flat[slot] = (dl & 127).astype(np.float32)
        # idx wrapped into 16 partitions; the gather ucode only reads
        # partitions 16:32 (probed on hw, deterministic across cores), so
        # upload just bands 0+1 (band 0 also covers the interp's read path)
        idx_all.append(np.tile(idx_flat.reshape(total // 16, 16).T, (2, 1)).copy())
        dv_all.append(dv_flat.reshape(total // 128, 128).T.astype(np.float16).copy())
    return env, slot_off, total, idx_all, dv_all


def _raw_dma_gather(gp, out_ap, in_ap, idxs_ap, num_idxs, elem_size, elem_step):
    """dma_gather without the elem_size_bytes%256 assert (transpose-path-only
    restriction). Rows are elem_step-strided; each descriptor moves elem_size
    elements. Verified bit-exact on hardware for fp16 elem 64 / step 128."""
    import concourse.mybir as mybir
    import concourse.ap_utils as ap_utils
    assert in_ap.dtype == out_ap.dtype
    assert idxs_ap.dtype == mybir.dt.int16
    assert ap_utils.ap_is_contiguous(in_ap.ap[1:])
    assert ap_utils.ap_is_contiguous(out_ap.ap[1:])
    assert ap_utils.ap_is_contiguous(idxs_ap.ap[1:])
    assert in_ap.ap[-1][1] == out_ap.ap[-1][1] == elem_size
    assert in_ap.ap[0][0] == elem_step
    assert out_ap.ap[0][1] * out_ap.ap[1][1] == _round_up(num_idxs, 128)
    stride_bytes = elem_step * mybir.dt.size(in_ap.dtype)
    assert stride_bytes % 256 == 0 and stride_bytes // 256 < 256
    _in_ap = gp.lower_ap_dma(in_ap, for_custom_bir_dma=True)
    return gp.add_instruction(
        mybir.InstDMAGatherAnt(
            name=gp.bass.get_next_instruction_name(),
            ins=[*_in_ap, gp.lower_ap(idxs_ap),
                 gp.lower_val_access(gp.to_reg(num_idxs))],
            outs=[gp.lower_ap(out_ap)],
            transpose=False,
            num_idxs=num_idxs,
            elem_size=elem_size,
            stride_bytes_256=stride_bytes // 256,
            gen_mode=0,
            single_packet=False,
            queue_num=0,
        )
    )


def _build_program(env, slot_off, total):
    import concourse.bass as bass
    import concourse.bacc as bacc
    import concourse.mybir as mybir
    import concourse.tile as tile

    f32 = mybir.dt.float32
    fp16 = mybir.dt.float16

    sup_slots = []
    for s in range(N_SUPER):
        n = sum(int(env[s * G + tl, c])
                for c in range(N_CHUNKS) for tl in range(G_OF[s]))
        sup_slots.append(n)
    sup_off = np.concatenate([[0], np.cumsum(sup_slots)])[:-1].astype(np.int64)
    max_sup_slots = _round_up(max(sup_slots), 2048)
    max_seg_blocks = max(
        sum(int(env[s * G + tl, c]) for tl in range(G_OF[s])) // 128
        for s in range(N_SUPER) for c in range(N_CHUNKS))

    nc = bacc.Bacc("TRN2", target_bir_lowering=False, debug=False,
                   num_devices=N_CORES)
    w_d = nc.dram_tensor("w", [N_NODES, 128], fp16, kind="ExternalInput").ap()
    ci_d = nc.dram_tensor("ci", [128, N_TILES], f32, kind="ExternalInput").ap()
    io_d = nc.dram_tensor("io", [128, 128], fp16, kind="ExternalInput").ap()
    idx_d = nc.dram_tensor("idx", [32, total // 16], mybir.dt.int16,
                           kind="ExternalInput").ap()
    dv_d = nc.dram_tensor("dv", [128, total // 128], fp16,
                          kind="ExternalInput").ap()
    out_d = nc.dram_tensor("out", [128, N_TILES * OUT_DIM], fp16,
                           kind="ExternalOutput").ap()

    with tile.TileContext(nc) as tc:
        with (
            tc.tile_pool(name="const", bufs=1) as constp,
            tc.tile_pool(name="idxp", bufs=4) as idxp,
            tc.tile_pool(name="dvp", bufs=4) as dvp,
            tc.tile_pool(name="msg", bufs=12) as msgp,
            tc.tile_pool(name="oh", bufs=3) as ohp,
            tc.tile_pool(name="ps", bufs=8, space="PSUM") as psp,
            tc.tile_pool(name="ot", bufs=3) as otp,
        ):
            ci_t = constp.tile([128, N_TILES], f32)
            io_t = constp.tile([128, 128], fp16)
            nc.sync.dma_start(ci_t[:], ci_d[:])
            nc.sync.dma_start(io_t[:], io_d[:])

            for s in range(N_SUPER):
                gs = G_OF[s]
                base = int(sup_off[s])
                nsl = sup_slots[s]
                # stage this supertile's metadata (rotating pools provide
                # back-pressure so uploads interleave with earlier gathers)
                idx_t = idxp.tile([128, max_sup_slots // 16], mybir.dt.int16,
                                  tag="idx", name=f"idx_{s}")
                dv_h = dvp.tile([128, max_sup_slots // 128], fp16,
                                tag="dvh", name=f"dvh_{s}")
                dv_t = dvp.tile([128, max_sup_slots // 128], f32,
                                tag="dv", name=f"dv_{s}")
                nc.sync.dma_start(idx_t[0:32, :nsl // 16],
                                  idx_d[:, base // 16:(base + nsl) // 16])
                nc.sync.dma_start(dv_h[:, :nsl // 128],
                                  dv_d[:, base // 128:(base + nsl) // 128])
                nc.scalar.activation(dv_t[:, :nsl // 128],
                                     dv_h[:, :nsl // 128],
                                     mybir.ActivationFunctionType.Copy)

                msgs = []
                rel = 0                              # slot offset within super
                seg_rel = []
                for c in range(N_CHUNKS):
                    n_sc = sum(int(env[s * G + tl, c]) for tl in range(gs))
                    msg = msgp.tile([128, max_seg_blocks, OUT_DIM], fp16,
                                    tag="msg", name=f"msg_{s}_{c}")
                    _raw_dma_gather(
                        nc.gpsimd, msg[:, :n_sc // 128, :],
                        w_d[CHUNK_BASE[c]:CHUNK_BASE[c] + CHUNK_SIZE[c],
                            0:OUT_DIM],
                        idx_t[:, rel // 16:(rel + n_sc) // 16],
                        n_sc, OUT_DIM, 128)
                    msgs.append(msg)
                    seg_rel.append(rel)
                    rel += n_sc

                n_blk_sup = nsl // 128
                oh_sup = ohp.tile([128, n_blk_sup, 128], fp16, tag="oh",
                                  name=f"oh_{s}", padded_shape=None)
                pss = [psp.tile([128, OUT_DIM], f32, tag="ps",
                                name=f"ps_{s}_{tl}") for tl in range(gs)]
                blk_total = [sum(int(env[s * G + tl, c])
                                 for c in range(N_CHUNKS)) // 128
                             for tl in range(gs)]
                blk_seen = [0] * gs
                for c in range(N_CHUNKS):
                    col = 0
                    for tl in range(gs):
                        n_blk_t = int(env[s * G + tl, c]) // 128
                        for b in range(n_blk_t):
                            gcol = (seg_rel[c] + 128 * (col + b)) // 128
                            nc.vector.tensor_scalar(
                                oh_sup[:, gcol, :], io_t[:],
                                dv_t[:, gcol:gcol + 1],
                                1.0, mybir.AluOpType.is_equal,
                                mybir.AluOpType.mult)
                            nc.tensor.matmul(
                                pss[tl][:], oh_sup[:, gcol, :],
                                msgs[c][:, col + b, :],
                                start=(blk_seen[tl] == 0),
                                stop=(blk_seen[tl] == blk_total[tl] - 1))
                            blk_seen[tl] += 1
                        col += n_blk_t

                ot = otp.tile([128, gs * OUT_DIM], fp16, tag="ot",
                              name=f"ot_{s}")
                for tl in range(gs):
                    t = s * G + tl
                    nc.scalar.activation(
                        ot[:, tl * OUT_DIM:(tl + 1) * OUT_DIM], pss[tl][:],
                        mybir.ActivationFunctionType.Copy,
                        scale=ci_t[:, t:t + 1])
                nc.sync.dma_start(
                    out_d[:, s * G * OUT_DIM:(s * G + gs) * OUT_DIM], ot[:])

    nc.compile()
    return nc


def prepare(node_ids, src_idx, dst_idx, cj, ci, weight):
    """Host prep + program build. Returns (nc, in_maps, postprocess)."""
    import time
    _t0 = time.time()

    node_ids = np.asarray(node_ids)
    src = np.asarray(src_idx).astype(np.int64)
    dst = np.asarray(dst_idx).astype(np.int64)
    cj = np.asarray(cj, dtype=np.float32).reshape(-1)
    ci = np.asarray(ci, dtype=np.float32).reshape(-1)
    weight = np.asarray(weight, dtype=np.float32)

    # feat rows are weight[node_ids]; with the arange fill this is identity
    if not np.array_equal(node_ids, np.arange(N_NODES, dtype=node_ids.dtype)):
        weight = weight[node_ids]

    # prescale by cj and lay out as an fp16 table with 256B-strided rows
    w_tab = np.zeros((N_NODES, 128), np.float16)
    w_tab[:, :OUT_DIM] = (weight * cj[:, None]).astype(np.float16)

    iota = np.tile(np.arange(128, dtype=np.float16), (128, 1))

    env, slot_off, total, idx_all, dv_all = _host_prep(src, dst)
    print(f"[kernel] host prep: {time.time()-_t0:.1f}s (total slots {total})",
          flush=True)
    _t1 = time.time()
    nc = _build_program(env, slot_off, total)
    print(f"[kernel] build+schedule+compile-to-bir: {time.time()-_t1:.1f}s",
          flush=True)

    in_maps = []
    for k in range(N_CORES):
        ci_k = np.zeros(N_TILES * 128, np.float32)
        ci_k[:DST_PER_CORE] = ci[k * DST_PER_CORE:(k + 1) * DST_PER_CORE]
        ci_w = ci_k.reshape(N_TILES, 128).T.copy()
        in_maps.append({
            "w": w_tab, "ci": ci_w, "io": iota,
            "idx": idx_all[k], "dv": dv_all[k],
        })

    def post(results):
        outs = []
        for k in range(N_CORES):
            o = np.asarray(results[k]["out"]).astype(np.float32)
            o = o.reshape(128, N_TILES, OUT_DIM).transpose(1, 0, 2)
            outs.append(o.reshape(-1, OUT_DIM)[:DST_PER_CORE])
        return np.concatenate(outs, axis=0)

    return nc, in_maps, post


def kernel(node_ids, src_idx, dst_idx, cj, ci, weight):
    import time
    from concourse.bass_utils import run_bass_kernel_spmd
    nc, in_maps, post = prepare(node_ids, src_idx, dst_idx, cj, ci, weight)
    _t2 = time.time()
    res = run_bass_kernel_spmd(nc, in_maps, core_ids=list(range(N_CORES)))
    print(f"[kernel] neff compile+exec: {time.time()-_t2:.1f}s", flush=True)
    return post(res.results)


# revision 14
# speedup vs baseline: 1.7279x; 1.0011x over previous
"""GCMC graph-conv kernel for Trainium2, 8-core SPMD.

out = ci * segment_sum((weight[node_ids] * cj)[src_idx], dst_idx)

Strategy (edge sharding by dst range, fp16 message path):
  - host prescales W' = weight[node_ids] * cj, stores it as an fp16 table with
    256B-strided rows ([100000, 128] fp16, data in cols 0:64) so each gather
    descriptor moves only 128B (half the DMA time of a 256B fp32 row)
  - core k owns dst rows [k*12500, (k+1)*12500); its edges are partitioned by
    (supertile of G=4 dst tiles, src chunk of 25000, dst tile) with each
    (supertile, chunk, tile) sub-segment padded to 128 slots using a shared
    static envelope (max over cores) so the program is SPMD-identical
  - one SWDGE dma_gather per (supertile, chunk) — 100 gathers instead of 392 —
    emitted raw (the bass wrapper's elem%256 assert is a transpose-path
    restriction; elem_step=128/elem_size=64 fp16 is valid and verified on hw)
  - segment-sum via one-hot matmul on DVE+PE: oh[slot, d] = (iota[d] ==
    dv[slot]); psum[dst, 64] += oh.T @ msg in fp16; psum groups are
    double-buffered (2 supertiles x 4 banks) so one supertile's tail overlaps
    the next's head
  - idx/dv metadata staged through rotating pools inside the supertile loop so
    uploads interleave with gathers instead of front-loading the DMA queue
  - flush: ACT copies psum*ci into a per-supertile staging tile, one HWDGE
    DMA per supertile writes [128, G*64] fp32 to a partition-major output
    buffer that the host untransposes
"""
import sys, os
sys.path.insert(0, '/opt/trn_rl_repo')

import numpy as np

N_NODES = 100000
OUT_DIM = 64
N_CORES = 8
DST_PER_CORE = N_NODES // N_CORES          # 12500
N_TILES = (DST_PER_CORE + 127) // 128      # 98
G = 4                                      # dst tiles per supertile
N_SUPER = (N_TILES + G - 1) // G           # 25 (last has 2 tiles)
G_OF = [min(G, N_TILES - s * G) for s in range(N_SUPER)]
# int16 gather indices allow chunks up to 32767 rows; uneven chunks put the
# per-(tile, chunk) 128-rounding waste into 3 big cells + 1 small one
CHUNK_BASE = [0, 32767, 65534, 98301]
CHUNK_SIZE = [32767, 32767, 32767, 1699]
N_CHUNKS = 4
PAD_SENTINEL = 999.0


def _round_up(x, m):
    return (x + m - 1) // m * m


def _host_prep(src, dst):
    """Partition edges by dst core range; compute the shared static envelope
    env[t, c] (max per-core (tile, chunk) count, rounded to 128) and per-core
    slot-packed idx / dv arrays laid out in envelope slots ordered by
    (supertile, chunk, tile)."""
    per_core = []
    counts = np.zeros((N_CORES, N_TILES, N_CHUNKS), np.int64)
    for k in range(N_CORES):
        m = (dst // DST_PER_CORE) == k
        s_e = src[m]
        dl = dst[m] - k * DST_PER_CORE
        t = dl >> 7                         # dst tile 0..97
        c = np.minimum(s_e // 32767, 3)     # src chunk 0..3
        counts[k] = np.bincount(t * N_CHUNKS + c,
                                minlength=N_TILES * N_CHUNKS).reshape(
                                    N_TILES, N_CHUNKS)
        per_core.append((s_e, dl, t, c))

    env = _round_up(counts.max(axis=0), 128)       # [T, C]
    # slot order: (supertile, chunk, tile)
    grp_key = []
    for s in range(N_SUPER):
        for c in range(N_CHUNKS):
            for tl in range(G_OF[s]):
                grp_key.append((s * G + tl, c))
    env_seq = np.array([env[t, c] for (t, c) in grp_key], np.int64)
    slot_off_seq = np.concatenate([[0], np.cumsum(env_seq)])[:-1]
    total = int(env_seq.sum())
    slot_off = np.zeros((N_TILES, N_CHUNKS), np.int64)
    for g, (t, c) in enumerate(grp_key):
        slot_off[t, c] = slot_off_seq[g]

    idx_all, dv_all = [], []
    for k in range(N_CORES):
        s_e, dl, t, c = per_core[k]
        gid = t * N_CHUNKS + c
        order = np.argsort(gid, kind='stable')
        s_e, dl, t, c, gid = (s_e[order], dl[order], t[order], c[order],
                              gid[order])
        gcounts = np.bincount(gid, minlength=N_TILES * N_CHUNKS)
        within = np.arange(len(s_e)) - np.repeat(
            np.concatenate([[0], np.cumsum(gcounts)])[:-1], gcounts)
        slot = slot_off[t, c] + within
        idx_flat = np.zeros(total, np.int16)       # pad slots gather row 0
        idx_flat[slot] = (s_e - np.asarray(CHUNK_BASE)[c]).astype(np.int16)
        dv_flat = np.full(total, PAD_SENTINEL, np.float32)
        dv_flat[slot] = (dl & 127).astype(np.float32)
        # idx wrapped into 16 partitions; the gather ucode only reads
        # partitions 16:32 (probed on hw, deterministic across cores), so
        # upload just bands 0+1 (band 0 also covers the interp's read path)
        idx_all.append(np.tile(idx_flat.reshape(total // 16, 16).T, (2, 1)).copy())
        dv_all.append(dv_flat.reshape(total // 128, 128).T.astype(np.float16).copy())
    return env, slot_off, total, idx_all, dv_all


def _raw_dma_gather(gp, out_ap, in_ap, idxs_ap, num_idxs, elem_size, elem_step):
    """dma_gather without the elem_size_bytes%256 assert (transpose-path-only
    restriction). Rows are elem_step-strided; each descriptor moves elem_size
    elements. Verified bit-exact on hardware for fp16 elem 64 / step 128."""
    import concourse.mybir as mybir
    import concourse.ap_utils as ap_utils
    assert in_ap.dtype == out_ap.dtype
    assert idxs_ap.dtype == mybir.dt.int16
    assert ap_utils.ap_is_contiguous(in_ap.ap[1:])
    assert ap_utils.ap_is_contiguous(out_ap.ap[1:])
    assert ap_utils.ap_is_contiguous(idxs_ap.ap[1:])
    assert in_ap.ap[-1][1] == out_ap.ap[-1][1] == elem_size
    assert in_ap.ap[0][0] == elem_step
    assert out_ap.ap[0][1] * out_ap.ap[1][1] == _round_up(num_idxs, 128)
    stride_bytes = elem_step * mybir.dt.size(in_ap.dtype)
    assert stride_bytes % 256 == 0 and stride_bytes // 256 < 256
    _in_ap = gp.lower_ap_dma(in_ap, for_custom_bir_dma=True)
    return gp.add_instruction(
        mybir.InstDMAGatherAnt(
            name=gp.bass.get_next_instruction_name(),
            ins=[*_in_ap, gp.lower_ap(idxs_ap),
                 gp.lower_val_access(gp.to_reg(num_idxs))],
            outs=[gp.lower_ap(out_ap)],
            transpose=False,
            num_idxs=num_idxs,
            elem_size=elem_size,
            stride_bytes_256=stride_bytes // 256,
            gen_mode=0,
            single_packet=False,
            queue_num=0,
        )
    )


def _build_program(env, slot_off, total):
    import concourse.bass as bass
    import concourse.bacc as bacc
    import concourse.mybir as mybir
    import concourse.tile as tile

    f32 = mybir.dt.float32
    fp16 = mybir.dt.float16

    sup_slots = []
    for s in range(N_SUPER):
        n = sum(int(env[s * G + tl, c])
                for c in range(N_CHUNKS) for tl in range(G_OF[s]))
        sup_slots.append(n)
    sup_off = np.concatenate([[0], np.cumsum(sup_slots)])[:-1].astype(np.int64)
    max_sup_slots = _round_up(max(sup_slots), 2048)
    max_seg_blocks = max(
        sum(int(env[s * G + tl, c]) for tl in range(G_OF[s])) // 128
        for s in range(N_SUPER) for c in range(N_CHUNKS))

    nc = bacc.Bacc("TRN2", target_bir_lowering=False, debug=False,
                   num_devices=N_CORES)
    w_d = nc.dram_tensor("w", [N_NODES, 128], fp16, kind="ExternalInput").ap()
    ci_d = nc.dram_tensor("ci", [128, N_TILES], f32, kind="ExternalInput").ap()
    io_d = nc.dram_tensor("io", [128, 128], fp16, kind="ExternalInput").ap()
    idx_d = nc.dram_tensor("idx", [32, total // 16], mybir.dt.int16,
                           kind="ExternalInput").ap()
    dv_d = nc.dram_tensor("dv", [128, total // 128], fp16,
                          kind="ExternalInput").ap()
    out_d = nc.dram_tensor("out", [128, N_TILES * OUT_DIM], fp16,
                           kind="ExternalOutput").ap()

    with tile.TileContext(nc) as tc:
        with (
            tc.tile_pool(name="const", bufs=1) as constp,
            tc.tile_pool(name="idxp", bufs=4) as idxp,
            tc.tile_pool(name="dvp", bufs=4) as dvp,
            tc.tile_pool(name="msg", bufs=12) as msgp,
            tc.tile_pool(name="oh", bufs=3) as ohp,
            tc.tile_pool(name="ps", bufs=8, space="PSUM") as psp,
            tc.tile_pool(name="ot", bufs=3) as otp,
        ):
            ci_t = constp.tile([128, N_TILES], f32)
            io_t = constp.tile([128, 128], fp16)

            for s in range(N_SUPER):
                gs = G_OF[s]
                base = int(sup_off[s])
                nsl = sup_slots[s]
                # stage this supertile's metadata (rotating pools provide
                # back-pressure so uploads interleave with earlier gathers);
                # idx is uploaded per chunk segment so the first gather only
                # waits on its own slice, and ci/io queue behind it
                idx_t = idxp.tile([128, max_sup_slots // 16], mybir.dt.int16,
                                  tag="idx", name=f"idx_{s}")
                dv_h = dvp.tile([128, max_sup_slots // 128], fp16,
                                tag="dvh", name=f"dvh_{s}")
                dv_t = dvp.tile([128, max_sup_slots // 128], f32,
                                tag="dv", name=f"dv_{s}")

                msgs = []
                rel = 0                              # slot offset within super
                seg_rel = []
                for c in range(N_CHUNKS):
                    n_sc = sum(int(env[s * G + tl, c]) for tl in range(gs))
                    nc.sync.dma_start(
                        idx_t[0:32, rel // 16:(rel + n_sc) // 16],
                        idx_d[:, (base + rel) // 16:(base + rel + n_sc) // 16])
                    if s == 0 and c == 0:
                        nc.sync.dma_start(ci_t[:], ci_d[:])
                        nc.sync.dma_start(io_t[:], io_d[:])
                        nc.sync.dma_start(
                            dv_h[:, :nsl // 128],
                            dv_d[:, base // 128:(base + nsl) // 128])
                        nc.scalar.activation(
                            dv_t[:, :nsl // 128], dv_h[:, :nsl // 128],
                            mybir.ActivationFunctionType.Copy)
                    msg = msgp.tile([128, max_seg_blocks, OUT_DIM], fp16,
                                    tag="msg", name=f"msg_{s}_{c}")
                    _raw_dma_gather(
                        nc.gpsimd, msg[:, :n_sc // 128, :],
                        w_d[CHUNK_BASE[c]:CHUNK_BASE[c] + CHUNK_SIZE[c],
                            0:OUT_DIM],
                        idx_t[:, rel // 16:(rel + n_sc) // 16],
                        n_sc, OUT_DIM, 128)
                    msgs.append(msg)
                    seg_rel.append(rel)
                    rel += n_sc
                if s > 0:
                    nc.sync.dma_start(
                        dv_h[:, :nsl // 128],
                        dv_d[:, base // 128:(base + nsl) // 128])
                    nc.scalar.activation(
                        dv_t[:, :nsl // 128], dv_h[:, :nsl // 128],
                        mybir.ActivationFunctionType.Copy)

                n_blk_sup = nsl // 128
                oh_sup = ohp.tile([128, n_blk_sup, 128], fp16, tag="oh",
                                  name=f"oh_{s}", padded_shape=None)
                pss = [psp.tile([128, OUT_DIM], f32, tag="ps",
                                name=f"ps_{s}_{tl}") for tl in range(gs)]
                blk_total = [sum(int(env[s * G + tl, c])
                                 for c in range(N_CHUNKS)) // 128
                             for tl in range(gs)]
                blk_seen = [0] * gs
                for c in range(N_CHUNKS):
                    col = 0
                    for tl in range(gs):
                        n_blk_t = int(env[s * G + tl, c]) // 128
                        for b in range(n_blk_t):
                            gcol = (seg_rel[c] + 128 * (col + b)) // 128
                            nc.vector.tensor_scalar(
                                oh_sup[:, gcol, :], io_t[:],
                                dv_t[:, gcol:gcol + 1],
                                1.0, mybir.AluOpType.is_equal,
                                mybir.AluOpType.mult)
                            nc.tensor.matmul(
                                pss[tl][:], oh_sup[:, gcol, :],
                                msgs[c][:, col + b, :],
                                start=(blk_seen[tl] == 0),
                                stop=(blk_seen[tl] == blk_total[tl] - 1))
                            blk_seen[tl] += 1
                        col += n_blk_t

                ot = otp.tile([128, gs * OUT_DIM], fp16, tag="ot",
                              name=f"ot_{s}")
                for tl in range(gs):
                    t = s * G + tl
                    nc.scalar.activation(
                        ot[:, tl * OUT_DIM:(tl + 1) * OUT_DIM], pss[tl][:],
                        mybir.ActivationFunctionType.Copy,
                        scale=ci_t[:, t:t + 1])
                nc.sync.dma_start(
                    out_d[:, s * G * OUT_DIM:(s * G + gs) * OUT_DIM], ot[:])

    nc.compile()
    return nc


def prepare(node_ids, src_idx, dst_idx, cj, ci, weight):
    """Host prep + program build. Returns (nc, in_maps, postprocess)."""
    import time
    _t0 = time.time()

    node_ids = np.asarray(node_ids)
    src = np.asarray(src_idx).astype(np.int64)
    dst = np.asarray(dst_idx).astype(np.int64)
    cj = np.asarray(cj, dtype=np.float32).reshape(-1)
    ci = np.asarray(ci, dtype=np.float32).reshape(-1)
    weight = np.asarray(weight, dtype=np.float32)

    # feat rows are weight[node_ids]; with the arange fill this is identity
    if not np.array_equal(node_ids, np.arange(N_NODES, dtype=node_ids.dtype)):
        weight = weight[node_ids]

    # prescale by cj and lay out as an fp16 table with 256B-strided rows
    w_tab = np.zeros((N_NODES, 128), np.float16)
    w_tab[:, :OUT_DIM] = (weight * cj[:, None]).astype(np.float16)

    iota = np.tile(np.arange(128, dtype=np.float16), (128, 1))

    env, slot_off, total, idx_all, dv_all = _host_prep(src, dst)
    print(f"[kernel] host prep: {time.time()-_t0:.1f}s (total slots {total})",
          flush=True)
    _t1 = time.time()
    nc = _build_program(env, slot_off, total)
    print(f"[kernel] build+schedule+compile-to-bir: {time.time()-_t1:.1f}s",
          flush=True)

    in_maps = []
    for k in range(N_CORES):
        ci_k = np.zeros(N_TILES * 128, np.float32)
        ci_k[:DST_PER_CORE] = ci[k * DST_PER_CORE:(k + 1) * DST_PER_CORE]
        ci_w = ci_k.reshape(N_TILES, 128).T.copy()
        in_maps.append({
            "w": w_tab, "ci": ci_w, "io": iota,
            "idx": idx_all[k], "dv": dv_all[k],
        })

    def post(results):
        outs = []
        for k in range(N_CORES):
            o = np.asarray(results[k]["out"]).astype(np.float32)
            o = o.reshape(128, N_TILES, OUT_DIM).transpose(1, 0, 2)
            outs.append(o.reshape(-1, OUT_DIM)[:DST_PER_CORE])
        return np.concatenate(outs, axis=0)

    return nc, in_maps, post


def kernel(node_ids, src_idx, dst_idx, cj, ci, weight):
    import time
    from concourse.bass_utils import run_bass_kernel_spmd
    nc, in_maps, post = prepare(node_ids, src_idx, dst_idx, cj, ci, weight)
    _t2 = time.time()
    res = run_bass_kernel_spmd(nc, in_maps, core_ids=list(range(N_CORES)))
    print(f"[kernel] neff compile+exec: {time.time()-_t2:.1f}s", flush=True)
    return post(res.results)
